# revision 24
# baseline (speedup 1.0000x reference)
"""Trainium2 Bass kernel for nn_DiffAtten (diffusion GNN + multi-head attention).

Model (per batch b): qc = LN([x; Ax; A^2x]) (L=3072 rows), vc likewise with v-graph;
MHA over L with H=4 heads of dim 16; o = attn-out @ w_fc + qc; LN; pool triples of
rows; conv+relu+linear+residual; final LN.  Output [2, 1024, 64] f32.

Sharding: 8 cores = 2 batches x 4 groups.  Core (b, g) computes attention for the
L-contiguous query chunk [768g, 768(g+1)) (covering output nodes [256g, 256(g+1))
after triple-pooling) against the full 3072-key side, recomputed on-core.

Numerics/engine strategy (validated against the f32 reference, ~4e-4 rel err):
  - scores and attn@V run as fp8e4 DoubleRow matmuls (0.5 PE cycles/row).
    Scores contract qc^T (fp8, feature-pairs on 32 partitions) against
    M_h-folded queries; attn@V contracts 17-wide (V|ones) blocks padded to
    32-partition lanes so denominators come out of the same accumulation.
  - exp runs with bias -2 (keeps e^s inside fp8e4 range) and is split across
    three engines: ACT uses the real activation table; DVE and Pool compute
    Schraudolph bit-space exp (one tensor_scalar each: u8 = round(s*c1+c2),
    bit-identical to an fp8 pattern; f32->u8 converts saturate, so the
    negative tail lands on +0.0 exactly).
  - everything else is bf16 (rows, transposes, weights) so DVE hits its
    2x 16-bit mode and PE transposes run 1 cycle/row; PSUM stays f32.
  - inputs stream in p-outer layout (node = p*8 + t) so every DMA is 128
    contiguous per-partition descriptors; A^T columns are host-permuted to
    match.  Adjacency DMAs issue from the ACT/DVE queues so all large
    transfers start at t=0 while SP issues the rest.
  - the pool->conv->linear epilogue stays in SBUF: triple-row pooling is
    three stride-3 PE matmuls against 0/1 selection matrices.
"""

import numpy as np

B, N, D = 2, 1024, 64
H, DK, DV = 4, 16, 16
DOUT = 128
STEPS = 3
L = STEPS * N          # 3072
P = 128
NT = N // P            # 8 node tiles
LT = L // P            # 24 L tiles
CH = L // 4            # 768 q-chunk per core
CN = N // 4            # 256 output nodes per core
QT3 = CH // 3          # 256 q columns per third
DV1 = DV + 1           # 17
RSQRT_MAGIC = 0x5F3759DF
C1_8 = 8.0 / np.log(2.0)           # fp8e4m3 Schraudolph slope
C2_8 = 7.0 * 8.0 - 2.0 * C1_8     # bias for exp(s - 2)

_CACHE = {}


def _bcast_ap(bass_mod, ap, parts):
    """[F] dram AP -> [parts, F] broadcast AP (partition step 0)."""
    return bass_mod.AP(tensor=ap.tensor, offset=ap.offset, ap=[[0, parts]] + list(ap.ap))


def _build_nc():
    import concourse.bass as bass
    import concourse.bacc as bacc
    import concourse.tile as tile
    from concourse import mybir, masks

    f32 = mybir.dt.float32
    i32 = mybir.dt.int32
    u8 = mybir.dt.uint8
    bf16 = mybir.dt.bfloat16
    fp8 = mybir.dt.float8e4
    AF = mybir.ActivationFunctionType
    OP = mybir.AluOpType
    PM = mybir.MatmulPerfMode

    nc = bacc.Bacc(None, target_bir_lowering=False)

    # ---- kernel I/O (per-core slices supplied by the host) ----
    xqb = nc.dram_tensor("xqb", [N, D], bf16, kind="ExternalInput")
    xvb = nc.dram_tensor("xvb", [N, D], bf16, kind="ExternalInput")
    atq = nc.dram_tensor("atq", [N, N], bf16, kind="ExternalInput")   # A^T, cols permuted
    atv = nc.dram_tensor("atv", [N, N], bf16, kind="ExternalInput")
    acq = nc.dram_tensor("acq", [N, CH], bf16, kind="ExternalInput")  # chunk operator^T
    m_pair = nc.dram_tensor("m_pair", [D, 2 * P], bf16, kind="ExternalInput")
    wv32 = nc.dram_tensor("wv32", [D + 1, P], bf16, kind="ExternalInput")
    wfc = nc.dram_tensor("wfc", [D, D], bf16, kind="ExternalInput")
    mha_w = nc.dram_tensor("mha_w", [D], bf16, kind="ExternalInput")
    mha_b = nc.dram_tensor("mha_b", [D], bf16, kind="ExternalInput")
    pq_w = nc.dram_tensor("pq_w", [D, 3 * D], bf16, kind="ExternalInput")
    conv_w3 = nc.dram_tensor("conv_w3", [D, DOUT], bf16, kind="ExternalInput")
    conv_b = nc.dram_tensor("conv_b", [DOUT], f32, kind="ExternalInput")
    lin_w = nc.dram_tensor("lin_w", [DOUT, D], bf16, kind="ExternalInput")
    lin_b = nc.dram_tensor("lin_b", [D], f32, kind="ExternalInput")
    norm_w = nc.dram_tensor("norm_w", [D], f32, kind="ExternalInput")
    norm_b = nc.dram_tensor("norm_b", [D], f32, kind="ExternalInput")
    rest = nc.dram_tensor("rest", [D, CN], f32, kind="ExternalInput")
    out_d = nc.dram_tensor("out", [CN, D], f32, kind="ExternalOutput")

    with tile.TileContext(nc) as tc:
        with (
            tc.tile_pool(name="consts", bufs=1) as consts,
            tc.tile_pool(name="big", bufs=1) as big,
            tc.tile_pool(name="tmp", bufs=4) as tmp,
            tc.tile_pool(name="ntmp", bufs=2) as ntmp,
        ):
            # ---------------- input DMAs ----------------
            # Big adjacency loads issue from ACT/DVE queues so their
            # transfers start immediately; SP issues the rest in
            # dependency order (chunk path first).
            xqb_sb = big.tile([P, NT, D], bf16)
            nc.sync.dma_start(xqb_sb[:, :, :], xqb[:, :].rearrange("(p t) d -> p t d", p=P))
            acq_sb = big.tile([P, NT, CH], bf16)
            acq_v = acq[:, :].rearrange("(p t) c -> p t c", p=P)
            nc.sync.dma_start(acq_sb[:, 0:4, :], acq_v[:, 0:4, :])
            nc.sync.dma_start(acq_sb[:, 4:NT, :], acq_v[:, 4:NT, :])
            atq_sb = big.tile([P, NT, N], bf16)
            nc.scalar.dma_start(atq_sb[:, :, :], atq[:, :].rearrange("(p t) i -> p t i", p=P))
            atv_sb = big.tile([P, NT, N], bf16)
            nc.sync.dma_start(atv_sb[:, :, :], atv[:, :].rearrange("(p t) i -> p t i", p=P))
            xvb_sb = big.tile([P, NT, D], bf16)
            nc.sync.dma_start(xvb_sb[:, :, :], xvb[:, :].rearrange("(p t) d -> p t d", p=P))
            m_sb = consts.tile([D, 2 * P], bf16)
            nc.sync.dma_start(m_sb[:, :], m_pair[:, :])
            wv_sb = consts.tile([D + 1, P], bf16)
            nc.sync.dma_start(wv_sb[:, :], wv32[:, :])
            wfc_sb = consts.tile([D, D], bf16)
            nc.sync.dma_start(wfc_sb[:, :], wfc[:, :])
            pq_sb = consts.tile([D, 3, D], bf16)
            nc.sync.dma_start(pq_sb[:, :, :], pq_w[:, :].rearrange("d (q e) -> d q e", q=3))
            convw_sb = consts.tile([D, DOUT], bf16)
            nc.sync.dma_start(convw_sb[:, :], conv_w3[:, :])
            convb_sb = consts.tile([DOUT, 1], f32)
            nc.sync.dma_start(convb_sb[:, :], conv_b[:].unsqueeze(1))
            linw_sb = consts.tile([DOUT, D], bf16)
            nc.sync.dma_start(linw_sb[:, :], lin_w[:, :])
            linb_sb = consts.tile([D, 1], f32)
            nc.sync.dma_start(linb_sb[:, :], lin_b[:].unsqueeze(1))
            rest_sb = consts.tile([D, CN], f32)
            nc.sync.dma_start(rest_sb[:, :], rest[:, :])
            mw_sb = consts.tile([P, D], bf16)
            nc.sync.dma_start(mw_sb[:, :], _bcast_ap(bass, mha_w[:], P))
            mb_sb = consts.tile([P, D], bf16)
            nc.sync.dma_start(mb_sb[:, :], _bcast_ap(bass, mha_b[:], P))
            nw_sb = consts.tile([P, D], f32)
            nc.sync.dma_start(nw_sb[:, :], _bcast_ap(bass, norm_w[:], P))
            nb_sb = consts.tile([P, D], f32)
            nc.sync.dma_start(nb_sb[:, :], _bcast_ap(bass, norm_b[:], P))

            # ---------------- constants ----------------
            idn = consts.tile([P, P], f32)
            masks.make_identity(nc, idn[:, :])
            idb = consts.tile([P, P], bf16)
            masks.make_identity(nc, idb[:, :])
            mneg2 = consts.tile([P, 1], f32)
            nc.gpsimd.memset(mneg2[:, :], -2.0)

            # persistent intermediates
            d_rows = {}   # (side, step) -> [128, 8, 64] bf16 rows of A^s x
            for side in ("q", "v"):
                for step in (1, 2):
                    d_rows[(side, step)] = big.tile(
                        [P, NT, D], bf16, tag=f"d{side}{step}", name=f"d{side}{step}")

            qrows = big.tile([P, LT, D], bf16)
            vrows = big.tile([P, LT, D], bf16)
            qcT8 = big.tile([D, L], fp8)              # qc^T (keys, fp8)
            vcT_bf = big.tile([D + 1, L], bf16)
            vr8 = big.tile([P, LT, P], fp8)           # (V|1) rows, heads at 32h
            qT8 = big.tile([D, H, CH], fp8)           # M_h qc_chunk^T (fp8)
            qcTc_bf = big.tile([D, CH], bf16)
            mv_q = big.tile([P, LT, 2], f32)
            mv_v = big.tile([P, LT, 2], f32)
            rs2 = big.tile([P, 2, LT], f32)           # rstd, dim1 = side
            oT_sb = big.tile([P, 2, 3, QT3], f32)     # attn out^T: (pair, third); head
                                                      # even at base 0, odd at base 64
            onr_sb = big.tile([P, 6, D], bf16)        # normalized attn out rows
            o2r_sb = big.tile([P, 6, D], f32)         # (o@wfc + qc) rows
            oln_sb = big.tile([P, 6, D], bf16)        # after mha_ln
            onT_sb = big.tile([D, 2, P], bf16)
            olnT_sb = big.tile([D, 3 * CN], bf16)
            zr = big.tile([P, 6, D], f32)
            mv2 = big.tile([P, 6, 2], f32)
            rst2 = big.tile([P, 6], f32)
            xT_sb = big.tile([D, CN], bf16)
            x1_sb = big.tile([DOUT, CN], bf16)
            x3T_sb = big.tile([D, CN], f32)
            xr_sb = big.tile([P, 2, D], f32)
            yout = big.tile([P, 2, D], f32)

            nc.gpsimd.memset(vcT_bf[D:D + 1, :], 1.0)   # ones row for denominators

            helper_rr = [0]   # round-robin counter for helper-engine work

            def hcopy(dst, src):
                """PSUM->SBUF casts: mostly DVE, every 3rd on ACT (Pool
                cannot read PSUM)."""
                helper_rr[0] += 1
                if helper_rr[0] % 3 == 0:
                    nc.scalar.copy(dst, src)
                else:
                    nc.vector.tensor_copy(dst, src)

            def rsqrt_newton(dst, src, shape, tag, iters=2, eng=None):
                """dst = 1/sqrt(src) via fast-inverse-sqrt + Newton (all on eng)."""
                e = eng or nc.vector
                hv = ntmp.tile(shape, f32, tag=tag + "h", name=tag + "h")
                e.tensor_scalar_mul(hv[:, :], src, 0.5)
                y = dst
                e.tensor_scalar(
                    out=y.bitcast(i32), in0=src.bitcast(i32),
                    scalar1=1, scalar2=None, op0=OP.logical_shift_right)
                e.tensor_scalar(
                    out=y.bitcast(i32), in0=y.bitcast(i32),
                    scalar1=-1, scalar2=None, op0=OP.bitwise_xor)
                e.tensor_scalar(
                    out=y.bitcast(i32), in0=y.bitcast(i32),
                    scalar1=RSQRT_MAGIC + 1, scalar2=None, op0=OP.add)
                t = ntmp.tile(shape, f32, tag=tag + "t", name=tag + "t")
                for _ in range(iters):
                    e.tensor_mul(t[:, :], y, y)
                    e.tensor_tensor(out=t[:, :], in0=t[:, :], in1=hv[:, :], op=OP.mult)
                    e.tensor_scalar(
                        out=t[:, :], in0=t[:, :], scalar1=-1.0, scalar2=1.5,
                        op0=OP.mult, op1=OP.add)
                    e.tensor_mul(y, y, t[:, :])

            def ln_grp(grp, tpool):
                """LN stats+apply+transposes for kt tiles of group grp, both
                sides.  Stats/newton/applies run on Pool (SBUF-only, engine
                otherwise idle); q-side stats stay on DVE for parallelism."""
                kts = list(range(grp * NT, (grp + 1) * NT))
                for srcs, mv in ((src_q, mv_q), (src_v, mv_v)):
                    for i in kts:
                        st = tmp.tile([P, 6], f32, tag="bnst")
                        nc.vector.bn_stats(st[:, :], srcs[i // NT][:, i % NT, :])
                        nc.vector.bn_aggr(mv[:, i, :], st[:, :])
                i0 = kts[0]
                ve = tmp.tile([P, 2, NT], f32, tag="ve")
                nc.vector.tensor_scalar_add(ve[:, 0, :], mv_q[:, i0:i0 + NT, 1], 1e-5)
                nc.vector.tensor_scalar_add(ve[:, 1, :], mv_v[:, i0:i0 + NT, 1], 1e-5)
                rsqrt_newton(rs2[:, :, i0:i0 + NT], ve[:, :, :], [P, 2, NT], "lng")
                for side_i, (srcs, rows, mv) in enumerate(
                        ((src_q, qrows, mv_q), (src_v, vrows, mv_v))):
                    for i in kts:
                        nc.vector.tensor_scalar(
                            out=rows[:, i, :], in0=srcs[i // NT][:, i % NT, :],
                            scalar1=mv[:, i, 0:1], scalar2=rs2[:, side_i, i:i + 1],
                            op0=OP.subtract, op1=OP.mult)
                # transposes + fp8/bf16 column copies
                for q0 in range(kts[0], kts[0] + NT, 4):
                    tpb = tpool.tile([D, 4, P], bf16, tag="tp", name="tpq")
                    for m in range(4):
                        nc.tensor.transpose(tpb[:, m, :], qrows[:, q0 + m, :], idb[:, :])
                    hcopy(qcT8[:, P * q0:P * (q0 + 4)],
                          tpb[:, :, :].rearrange("d m p -> d (m p)"))
                for q0 in range(kts[0], kts[0] + NT, 4):
                    tpb = tpool.tile([D, 4, P], bf16, tag="tp", name="tpv")
                    for m in range(4):
                        nc.tensor.transpose(tpb[:, m, :], vrows[:, q0 + m, :], idb[:, :])
                    nc.vector.tensor_copy(
                        vcT_bf[0:D, P * q0:P * (q0 + 4)],
                        tpb[:, :, :].rearrange("d m p -> d (m p)"))
                # V rows (heads padded to 32 lanes, ones col at 32h+16)
                for q0 in range(kts[0], kts[0] + NT, 4):
                    vps = tpool.tile([P, 4, P], f32, tag="tp", name="vps")
                    for m in range(4):
                        nc.tensor.matmul(vps[:, m, :],
                                         lhsT=vcT_bf[:, P * (q0 + m):P * (q0 + m + 1)],
                                         rhs=wv_sb[:, :], start=True, stop=True)
                    hcopy(vr8[:, q0:q0 + 4, :], vps[:, :, :])

            # ---- attention inner iteration ----
            exp_sched = [0]
            prev_ex = [None]
            av_state = {}

            def av_emit(pair, heads, expair):
                avs = av_state["avs"]
                for h in heads:
                    nc.tensor.matmul(
                        avs[h][:, :],
                        lhsT=vr8[:, 2 * pair:2 * pair + 2, 32 * h:32 * h + 32],
                        rhs=expair[:, :, h, :],
                        start=(pair == 0), stop=(pair == LT // 2 - 1),
                        perf_mode=PM.DoubleRow, skip_group_check=True)

            def attn_iter(t3, kt, scp, expair):
                # scores per head-pair (one PSUM bank each) so exp can run
                # at 512-col granularity on alternating engines
                for pp in range(2):
                    sc = scp.tile([P, 2, QT3], f32, tag=f"sc{pp}")
                    nc.tensor.matmul(
                        sc[:, :, :],
                        lhsT=qcT8[:, P * kt:P * (kt + 1)],
                        rhs=qT8[:, 2 * pp:2 * pp + 2, QT3 * t3:QT3 * (t3 + 1)],
                        start=True, stop=True)
                    exd = expair[:, kt % 2, 2 * pp:2 * pp + 2, :]
                    c = exp_sched[0]
                    exp_sched[0] += 1
                    if c % 3 < 2 or c >= 128:
                        nc.scalar.activation(exd, sc[:, :, :], AF.Exp,
                                             bias=mneg2[:, :], scale=1.0)
                    else:
                        nc.vector.tensor_scalar(
                            out=exd.bitcast(u8), in0=sc[:, :, :],
                            scalar1=C1_8, scalar2=C2_8, op0=OP.mult, op1=OP.add)
                # spread the pair's 4 attn@V matmuls across two kt slots so
                # the PE has filler work while exp(kt) completes (keeps the
                # in-order PE from stalling and dropping out of max p-state)
                if kt % 2 == 1:
                    av_emit(kt // 2, (0, 1), expair)
                    if kt == LT - 1:
                        av_emit(kt // 2, (2, 3), expair)
                elif kt > 0:
                    av_emit(kt // 2 - 1, (2, 3), prev_ex[0])
                prev_ex[0] = expair

            def o_chain(t3, tpool):
                """Normalize o~ by softmax denominators, apply w_fc + residual +
                mha_ln, build olnT columns for the pooling matmuls."""
                for h in range(H):
                    pair, b = h // 2, 64 * (h % 2)
                    for jj in range(2):
                        j = 2 * t3 + jj
                        tpo = tpool.tile([P, DV1], f32, tag="tp", name="tpo")
                        nc.tensor.transpose(
                            tpo[:, :],
                            oT_sb[b:b + DV1, pair, t3, P * jj:P * (jj + 1)],
                            idn[b:b + DV1, b:b + DV1],
                            tile_position=(b, 0))
                        rec = tmp.tile([P, 1], f32, tag="rec")
                        nc.vector.reciprocal(rec[:, :], tpo[:, DV:DV1])
                        nc.vector.tensor_scalar_mul(
                            onr_sb[:, j, DV * h:DV * (h + 1)], tpo[:, 0:DV], rec[:, :])
                for jj in range(2):
                    j = 2 * t3 + jj
                    tpn = tpool.tile([D, P], bf16, tag="tp", name="tpn")
                    nc.tensor.transpose(tpn[:, :], onr_sb[:, j, :], idb[:, :])
                    nc.vector.tensor_copy(onT_sb[:, jj, :], tpn[:, :])
                    o2p = tpool.tile([P, D], f32, tag="tp", name="o2p")
                    nc.tensor.matmul(o2p[:, :], lhsT=onT_sb[:, jj, :],
                                     rhs=wfc_sb[:, :], start=True, stop=True)
                    nc.vector.tensor_add(o2r_sb[:, j, :], o2p[:, :], zr[:, j, :])
                    st2 = tmp.tile([P, 6], f32, tag="bnst2")
                    nc.vector.bn_stats(st2[:, :], o2r_sb[:, j, :])
                    nc.vector.bn_aggr(mv2[:, j, :], st2[:, :])
                ve2 = tmp.tile([P, 2], f32, tag="ve2")
                nc.vector.tensor_scalar_add(ve2[:, :], mv2[:, 2 * t3:2 * t3 + 2, 1], 1e-6)
                rsqrt_newton(rst2[:, 2 * t3:2 * t3 + 2], ve2[:, :], [P, 2], "ml")
                for jj in range(2):
                    j = 2 * t3 + jj
                    nc.vector.tensor_scalar(
                        out=oln_sb[:, j, :], in0=o2r_sb[:, j, :],
                        scalar1=mv2[:, j, 0:1], scalar2=rst2[:, j:j + 1],
                        op0=OP.subtract, op1=OP.mult)
                    nc.gpsimd.tensor_mul(oln_sb[:, j, :], oln_sb[:, j, :], mw_sb[:, :])
                    nc.gpsimd.tensor_add(oln_sb[:, j, :], oln_sb[:, j, :], mb_sb[:, :])
                    tpl = tpool.tile([D, P], bf16, tag="tp", name="tpl")
                    nc.tensor.transpose(tpl[:, :], oln_sb[:, j, :], idb[:, :])
                    nc.vector.tensor_copy(olnT_sb[:, P * j:P * (j + 1)], tpl[:, :])

            src_q = [xqb_sb, d_rows[("q", 1)], d_rows[("q", 2)]]
            src_v = [xvb_sb, d_rows[("v", 1)], d_rows[("v", 2)]]

            with (
                tc.tile_pool(name="tp", bufs=2, space="PSUM") as tp_pool,
            ):
                # ===== chunk path: z = A_chunk x (feature-major), LN row-wise =====
                with tc.tile_pool(name="chk", bufs=1, space="PSUM") as chk:
                    zps = chk.tile([D, CH], f32, tag="zps")
                    for t in range(NT):
                        nc.tensor.matmul(zps[:, 0:512], lhsT=xqb_sb[:, t, :],
                                         rhs=acq_sb[:, t, 0:512], start=(t == 0), stop=(t == NT - 1))
                        nc.tensor.matmul(zps[:, 512:CH], lhsT=xqb_sb[:, t, :],
                                         rhs=acq_sb[:, t, 512:CH], start=(t == 0), stop=(t == NT - 1))
                    zT = tmp.tile([D, CH], bf16, tag="zT")
                    nc.vector.tensor_copy(zT[:, :], zps[:, :])
                    for j0, cnt in ((0, 4), (4, 2)):
                        tq = tp_pool.tile([P, 4, D], bf16, tag="tp", name="ztq")
                        for m in range(cnt):
                            nc.tensor.transpose(tq[:, m, :], zT[:, P * (j0 + m):P * (j0 + m + 1)],
                                                idb[:D, :D])
                        nc.vector.tensor_copy(zr[:, j0:j0 + cnt, :], tq[:, 0:cnt, :])
                    mvc = tmp.tile([P, 6, 2], f32, tag="mvc")
                    for j in range(6):
                        stc = tmp.tile([P, 6], f32, tag="bnst")
                        nc.vector.bn_stats(stc[:, :], zr[:, j, :])
                        nc.vector.bn_aggr(mvc[:, j, :], stc[:, :])
                    vec = tmp.tile([P, 6], f32, tag="vec")
                    nc.vector.tensor_scalar_add(vec[:, :], mvc[:, :, 1], 1e-5)
                    rsc = tmp.tile([P, 6], f32, tag="rsc")
                    rsqrt_newton(rsc[:, :], vec[:, :], [P, 6], "chk")
                    zq = tmp.tile([P, 6, D], bf16, tag="zq")
                    for j in range(6):
                        nc.vector.tensor_scalar(
                            out=zr[:, j, :], in0=zr[:, j, :],
                            scalar1=mvc[:, j, 0:1], scalar2=rsc[:, j:j + 1],
                            op0=OP.subtract, op1=OP.mult)
                        nc.vector.tensor_copy(zq[:, j, :], zr[:, j, :])
                    for j0, cnt in ((0, 4), (4, 2)):
                        tq2 = tp_pool.tile([D, 4, P], bf16, tag="tp", name="ztq2")
                        for m in range(cnt):
                            nc.tensor.transpose(tq2[:, m, :], zq[:, j0 + m, :], idb[:, :])
                        nc.vector.tensor_copy(
                            qcTc_bf[:, P * j0:P * (j0 + cnt)],
                            tq2[:, 0:cnt, :].rearrange("d m p -> d (m p)"))
                    # q~ per head pair: rows 0-63 = head 2p, 64-127 = head 2p+1
                    for pp in range(2):
                        qps = chk.tile([P, CH], f32, tag="sb", name="qps")
                        nc.tensor.matmul(qps[:, 0:512], lhsT=m_sb[:, P * pp:P * (pp + 1)],
                                         rhs=qcTc_bf[:, 0:512], start=True, stop=True)
                        nc.tensor.matmul(qps[:, 512:CH], lhsT=m_sb[:, P * pp:P * (pp + 1)],
                                         rhs=qcTc_bf[:, 512:CH], start=True, stop=True)
                        hcopy(qT8[:, 2 * pp, :], qps[0:D, :])
                        hcopy(qT8[:, 2 * pp + 1, :], qps[D:P, :])

                # ===== step-0 layernorm (kt 0..7 both sides) + V rows =====
                ln_grp(0, tp_pool)

                # ===== diffusion task list, interleaved with attention =====
                def diffuse_tile(at_sb, lhs_src, dst, i):
                    dps = tp_pool.tile([P, D], f32, tag="tp", name="dps")
                    for j in range(NT):
                        nc.tensor.matmul(
                            dps[:, :], lhsT=at_sb[:, j, P * i:P * (i + 1)],
                            rhs=lhs_src[:, j, :],
                            start=(j == 0), stop=(j == NT - 1))
                    hcopy(dst[:, i, :], dps[:, :])

                diff_tasks = []
                for at_sb_, lhs_, dst_ in (
                    (atq_sb, xqb_sb, d_rows[("q", 1)]),
                    (atv_sb, xvb_sb, d_rows[("v", 1)]),
                    (atq_sb, d_rows[("q", 1)], d_rows[("q", 2)]),
                    (atv_sb, d_rows[("v", 1)], d_rows[("v", 2)]),
                ):
                    for i_ in range(NT):
                        diff_tasks.append((at_sb_, lhs_, dst_, i_))
                diff_tasks = diff_tasks[::-1]  # pop from the end

                def emit_diff(n):
                    for _ in range(n):
                        if diff_tasks:
                            diffuse_tile(*diff_tasks.pop())

                with (
                    tc.tile_pool(name="psE", bufs=1, space="PSUM") as psE,
                    tc.tile_pool(name="psEa", bufs=1, space="PSUM") as psEa,
                    tc.tile_pool(name="expp", bufs=3) as expp,
                ):
                    def av_tiles(t3):
                        return [psEa.tile([32, QT3], f32, tag=f"avP{h}", name=f"av{t3}{h}")
                                for h in range(H)]

                    def flush(t3, avs):
                        for h in range(H):
                            nc.vector.tensor_copy(
                                oT_sb[64 * (h % 2):64 * (h % 2) + 32, h // 2, t3, :],
                                avs[h][:, :])

                    avs = av_tiles(0)
                    av_state["avs"] = avs
                    expair = None
                    emit_diff(12)
                    for grp in range(3):
                        if grp > 0:
                            ln_grp(grp, tp_pool)
                        for kt in range(grp * NT, (grp + 1) * NT):
                            if kt % 2 == 0:
                                expair = expp.tile([P, 2, H, QT3], fp8, tag="ex")
                            attn_iter(0, kt, psE, expair)
                            emit_diff(1 if kt < NT else 2)
                    emit_diff(32)
                    flush(0, avs)
                    o_chain(0, tp_pool)
                    for t3 in (1, 2):
                        avs = av_tiles(t3)
                        av_state["avs"] = avs
                        for kt in range(LT):
                            if kt % 2 == 0:
                                expair = expp.tile([P, 2, H, QT3], fp8, tag="ex")
                            attn_iter(t3, kt, psE, expair)
                        flush(t3, avs)
                        if t3 == 1:
                            o_chain(1, tp_pool)

            # ================= epilogue =================
            with (
                tc.tile_pool(name="psF", bufs=4, space="PSUM") as psF,
                tc.tile_pool(name="psFf", bufs=1, space="PSUM") as psFf,
            ):
                o_chain(2, psF)
                # pooling: xT[d, n] = sum_q Pq^T olnT[:, 3n+q]
                xps = psFf.tile([D, CN], f32, tag="xps")
                olv = olnT_sb[:, :].rearrange("d (j s) -> d s j", s=3)
                for q in range(3):
                    nc.tensor.matmul(xps[:, :], lhsT=pq_sb[:, q, :], rhs=olv[:, q, :],
                                     start=(q == 0), stop=(q == 2))
                nc.vector.tensor_copy(xT_sb[:, :], xps[:, :])
                # conv/relu/lin/residual (1/3 pool-mean folded into conv_w3)
                x1ps = psFf.tile([DOUT, CN], f32, tag="x1ps")
                nc.tensor.matmul(x1ps[:, :], lhsT=convw_sb[:, :], rhs=xT_sb[:, :],
                                 start=True, stop=True)
                nc.scalar.activation(x1_sb[:, :], x1ps[:, :], AF.Relu,
                                     bias=convb_sb[:, :], scale=1.0)
                x2ps = psFf.tile([D, CN], f32, tag="x2ps")
                nc.tensor.matmul(x2ps[:, :], lhsT=linw_sb[:, :], rhs=x1_sb[:, :],
                                 start=True, stop=True)
                nc.vector.tensor_scalar_add(x3T_sb[:, :], x2ps[:, :], linb_sb[:, :])
                nc.vector.tensor_add(x3T_sb[:, :], x3T_sb[:, :], rest_sb[:, :])
                # rows + final LN (affine, eps 1e-5)
                for n2 in range(2):
                    tpf = psF.tile([P, D], f32, tag="tp")
                    nc.tensor.transpose(tpf[:, :], x3T_sb[:, P * n2:P * (n2 + 1)], idn[:D, :D])
                    nc.vector.tensor_copy(xr_sb[:, n2, :], tpf[:, :])
                mv3 = tmp.tile([P, 2, 2], f32, tag="mv3")
                for n2 in range(2):
                    st3 = tmp.tile([P, 6], f32, tag="bnst3")
                    nc.vector.bn_stats(st3[:, :], xr_sb[:, n2, :])
                    nc.vector.bn_aggr(mv3[:, n2, :], st3[:, :])
                ve3 = tmp.tile([P, 2], f32, tag="ve3")
                nc.vector.tensor_scalar_add(ve3[:, :], mv3[:, :, 1], 1e-5)
                rst3 = tmp.tile([P, 2], f32, tag="rst3")
                rsqrt_newton(rst3[:, :], ve3[:, :], [P, 2], "fl")
                for n2 in range(2):
                    nc.vector.tensor_scalar(
                        out=yout[:, n2, :], in0=xr_sb[:, n2, :],
                        scalar1=mv3[:, n2, 0:1], scalar2=rst3[:, n2:n2 + 1],
                        op0=OP.subtract, op1=OP.mult)
                    nc.gpsimd.tensor_mul(yout[:, n2, :], yout[:, n2, :], nw_sb[:, :])
                    nc.gpsimd.tensor_add(yout[:, n2, :], yout[:, n2, :], nb_sb[:, :])
                nc.sync.dma_start(out_d[:, :].rearrange("(t p) d -> p t d", p=P), yout[:, :, :])

    nc.finalize()
    return nc


def _prep_in_maps(inputs):
    import ml_dtypes
    bf = ml_dtypes.bfloat16

    q_x = np.asarray(inputs["q_x"], np.float32)
    v_x = np.asarray(inputs["v_x"], np.float32)
    q_adj = np.asarray(inputs["q_adj"], np.float32)
    v_adj = np.asarray(inputs["v_adj"], np.float32)
    w_qs = np.asarray(inputs["w_qs"], np.float32)
    w_ks = np.asarray(inputs["w_ks"], np.float32)
    w_vs = np.asarray(inputs["w_vs"], np.float32)
    w_fc = np.asarray(inputs["w_fc"], np.float32)
    mha_ln_w = np.asarray(inputs["mha_ln_w"], np.float32)
    mha_ln_b = np.asarray(inputs["mha_ln_b"], np.float32)
    conv_w = np.asarray(inputs["conv_w"], np.float32)
    conv_b = np.asarray(inputs["conv_b"], np.float32)
    lin_w = np.asarray(inputs["lin_w"], np.float32)
    lin_b = np.asarray(inputs["lin_b"], np.float32)
    norm_w = np.asarray(inputs["norm_w"], np.float32)
    norm_b = np.asarray(inputs["norm_b"], np.float32)

    # M_h = (Wq_h @ Wk_h^T) / sqrt(DK), head pairs side by side
    m_pair = np.zeros((D, 2 * P), np.float32)
    for h in range(H):
        m_pair[:, D * h:D * (h + 1)] = (
            w_qs[:, DK * h:DK * (h + 1)] @ w_ks[:, DK * h:DK * (h + 1)].T
        ) / np.sqrt(DK)
    # V projection: head h at cols 32h..32h+16 (16 V cols + ones col)
    wv32 = np.zeros((D + 1, P), np.float32)
    for h in range(H):
        wv32[:D, 32 * h:32 * h + DV] = w_vs[:, DV * h:DV * (h + 1)]
        wv32[D, 32 * h + DV] = 1.0
    # pooling selectors: out(n,d) = sum_q sum_c Pq[c,d] oln[3n+q, c]
    pq_w = np.zeros((D, 3, D), np.float32)
    for d in range(D):
        for s in range(STEPS):
            q, c = divmod(3 * d + s, D)
            pq_w[c, q, d] = 1.0
    conv_w3 = conv_w / 3.0

    # column permutation for A^T: col' i*128+m holds original node m*8+i
    idx = np.arange(N)
    colperm = (idx % P) * NT + idx // P

    shared = dict(
        m_pair=m_pair.astype(bf),
        wv32=wv32.astype(bf),
        wfc=w_fc.astype(bf),
        mha_w=mha_ln_w.astype(bf), mha_b=mha_ln_b.astype(bf),
        pq_w=pq_w.reshape(D, 3 * D).astype(bf),
        conv_w3=conv_w3.astype(bf), conv_b=conv_b,
        lin_w=lin_w.astype(bf), lin_b=lin_b,
        norm_w=norm_w, norm_b=norm_b,
    )

    per_batch = []
    for b in range(B):
        A, Av = q_adj[b], v_adj[b]
        A2 = A @ A
        G = np.concatenate([np.eye(N, dtype=np.float32), A, A2], axis=0)  # [3N, N]
        per_batch.append(dict(
            xqb=q_x[b].astype(bf), xvb=v_x[b].astype(bf),
            atq=np.ascontiguousarray(A.T[:, colperm]).astype(bf),
            atv=np.ascontiguousarray(Av.T[:, colperm]).astype(bf),
            G=G,
        ))

    in_maps = []
    for c in range(8):
        b, g = c // 4, c % 4
        pb = per_batch[b]
        acq = np.ascontiguousarray(pb["G"][CH * g:CH * (g + 1)].T).astype(bf)  # [N, CH]
        rest = np.ascontiguousarray(q_x[b, CN * g:CN * (g + 1)].T)             # [D, CN]
        m = dict(shared)
        m.update(xqb=pb["xqb"], xvb=pb["xvb"],
                 atq=pb["atq"], atv=pb["atv"], acq=acq, rest=rest)
        in_maps.append(m)
    return in_maps


def _run(inputs, trace=False, **kw):
    from concourse.bass_utils import run_bass_kernel_spmd

    if "nc" not in _CACHE:
        _CACHE["nc"] = _build_nc()
    nc = _CACHE["nc"]
    in_maps = _prep_in_maps(inputs)
    res = run_bass_kernel_spmd(nc, in_maps, core_ids=list(range(8)), trace=trace, **kw)
    out = np.empty((B, N, D), np.float32)
    for c in range(8):
        b, g = c // 4, c % 4
        out[b, CN * g:CN * (g + 1)] = res.results[c]["out"]
    return out, res


def kernel(**inputs) -> np.ndarray:
    out, _ = _run(inputs, trace=False)
    return out


# revision 25
# speedup vs baseline: 1.1590x; 1.1590x over previous
"""Trainium2 Bass kernel for nn_DiffAtten (diffusion GNN + multi-head attention).

Model (per batch b): qc = LN([x; Ax; A^2x]) (L=3072 rows), vc likewise with v-graph;
MHA over L with H=4 heads of dim 16; o = attn-out @ w_fc + qc; LN; pool triples of
rows; conv+relu+linear+residual; final LN.  Output [2, 1024, 64] f32.

Sharding: 8 cores = 2 batches x 4 groups.  Core (b, g) computes attention for the
L-contiguous query chunk [768g, 768(g+1)) (covering output nodes [256g, 256(g+1))
after triple-pooling) against the full 3072-key side, recomputed on-core.

Numerics/engine strategy (validated against the f32 reference, ~4e-4 rel err):
  - scores and attn@V run as fp8e4 DoubleRow matmuls (0.5 PE cycles/row).
    Scores contract qc^T (fp8, feature-pairs on 32 partitions) against
    M_h-folded queries; attn@V contracts 17-wide (V|ones) blocks padded to
    32-partition lanes so denominators come out of the same accumulation.
  - exp runs with bias -2 (keeps e^s inside fp8e4 range) and is split across
    three engines: ACT uses the real activation table; DVE and Pool compute
    Schraudolph bit-space exp (one tensor_scalar each: u8 = round(s*c1+c2),
    bit-identical to an fp8 pattern; f32->u8 converts saturate, so the
    negative tail lands on +0.0 exactly).
  - everything else is bf16 (rows, transposes, weights) so DVE hits its
    2x 16-bit mode and PE transposes run 1 cycle/row; PSUM stays f32.
  - inputs stream in p-outer layout (node = p*8 + t) so every DMA is 128
    contiguous per-partition descriptors; A^T columns are host-permuted to
    match.  Adjacency DMAs issue from the ACT/DVE queues so all large
    transfers start at t=0 while SP issues the rest.
  - the pool->conv->linear epilogue stays in SBUF: triple-row pooling is
    three stride-3 PE matmuls against 0/1 selection matrices.
"""

import numpy as np

B, N, D = 2, 1024, 64
H, DK, DV = 4, 16, 16
DOUT = 128
STEPS = 3
L = STEPS * N          # 3072
P = 128
NT = N // P            # 8 node tiles
LT = L // P            # 24 L tiles
CH = L // 4            # 768 q-chunk per core
CN = N // 4            # 256 output nodes per core
QT3 = CH // 3          # 256 q columns per third
DV1 = DV + 1           # 17
RSQRT_MAGIC = 0x5F3759DF
C1_8 = 8.0 / np.log(2.0)           # fp8e4m3 Schraudolph slope
C2_8 = 7.0 * 8.0 - 2.0 * C1_8     # bias for exp(s - 2)

_CACHE = {}


def _bcast_ap(bass_mod, ap, parts):
    """[F] dram AP -> [parts, F] broadcast AP (partition step 0)."""
    return bass_mod.AP(tensor=ap.tensor, offset=ap.offset, ap=[[0, parts]] + list(ap.ap))


def _build_nc():
    import concourse.bass as bass
    import concourse.bacc as bacc
    import concourse.tile as tile
    from concourse import mybir, masks

    f32 = mybir.dt.float32
    i32 = mybir.dt.int32
    u8 = mybir.dt.uint8
    bf16 = mybir.dt.bfloat16
    fp8 = mybir.dt.float8e4
    AF = mybir.ActivationFunctionType
    OP = mybir.AluOpType
    PM = mybir.MatmulPerfMode

    nc = bacc.Bacc(None, target_bir_lowering=False)

    # ---- kernel I/O (per-core slices supplied by the host) ----
    xqb = nc.dram_tensor("xqb", [N, D], bf16, kind="ExternalInput")
    xvb = nc.dram_tensor("xvb", [N, D], bf16, kind="ExternalInput")
    atq = nc.dram_tensor("atq", [N, N], bf16, kind="ExternalInput")   # A^T, cols permuted
    atv = nc.dram_tensor("atv", [N, N], bf16, kind="ExternalInput")
    acq = nc.dram_tensor("acq", [N, CH], bf16, kind="ExternalInput")  # chunk operator^T
    m_pair = nc.dram_tensor("m_pair", [D, 2 * P], bf16, kind="ExternalInput")
    wv32 = nc.dram_tensor("wv32", [D + 1, P], bf16, kind="ExternalInput")
    wfc = nc.dram_tensor("wfc", [D, D], bf16, kind="ExternalInput")
    mha_w = nc.dram_tensor("mha_w", [D], bf16, kind="ExternalInput")
    mha_b = nc.dram_tensor("mha_b", [D], bf16, kind="ExternalInput")
    pq_w = nc.dram_tensor("pq_w", [D, 3 * D], bf16, kind="ExternalInput")
    conv_w3 = nc.dram_tensor("conv_w3", [D, DOUT], bf16, kind="ExternalInput")
    conv_b = nc.dram_tensor("conv_b", [DOUT], f32, kind="ExternalInput")
    lin_w = nc.dram_tensor("lin_w", [DOUT, D], bf16, kind="ExternalInput")
    lin_b = nc.dram_tensor("lin_b", [D], f32, kind="ExternalInput")
    norm_w = nc.dram_tensor("norm_w", [D], f32, kind="ExternalInput")
    norm_b = nc.dram_tensor("norm_b", [D], f32, kind="ExternalInput")
    rest = nc.dram_tensor("rest", [D, CN], f32, kind="ExternalInput")
    out_d = nc.dram_tensor("out", [CN, D], f32, kind="ExternalOutput")

    with tile.TileContext(nc) as tc:
        with (
            tc.tile_pool(name="consts", bufs=1) as consts,
            tc.tile_pool(name="big", bufs=1) as big,
            tc.tile_pool(name="tmp", bufs=4) as tmp,
            tc.tile_pool(name="ntmp", bufs=2) as ntmp,
        ):
            # ---------------- input DMAs ----------------
            # Big adjacency loads issue from ACT/DVE queues so their
            # transfers start immediately; SP issues the rest in
            # dependency order (chunk path first).
            xqb_sb = big.tile([P, NT, D], bf16)
            nc.sync.dma_start(xqb_sb[:, :, :], xqb[:, :].rearrange("(p t) d -> p t d", p=P))
            acq_sb = big.tile([P, NT, CH], bf16)
            acq_v = acq[:, :].rearrange("(p t) c -> p t c", p=P)
            nc.sync.dma_start(acq_sb[:, 0:4, :], acq_v[:, 0:4, :])
            nc.sync.dma_start(acq_sb[:, 4:NT, :], acq_v[:, 4:NT, :])
            atq_sb = big.tile([P, NT, N], bf16)
            nc.scalar.dma_start(atq_sb[:, :, :], atq[:, :].rearrange("(p t) i -> p t i", p=P))
            atv_sb = big.tile([P, NT, N], bf16)
            nc.sync.dma_start(atv_sb[:, :, :], atv[:, :].rearrange("(p t) i -> p t i", p=P))
            xvb_sb = big.tile([P, NT, D], bf16)
            nc.sync.dma_start(xvb_sb[:, :, :], xvb[:, :].rearrange("(p t) d -> p t d", p=P))
            m_sb = consts.tile([D, 2 * P], bf16)
            nc.sync.dma_start(m_sb[:, :], m_pair[:, :])
            wv_sb = consts.tile([D + 1, P], bf16)
            nc.sync.dma_start(wv_sb[:, :], wv32[:, :])
            wfc_sb = consts.tile([D, D], bf16)
            nc.sync.dma_start(wfc_sb[:, :], wfc[:, :])
            pq_sb = consts.tile([D, 3, D], bf16)
            nc.sync.dma_start(pq_sb[:, :, :], pq_w[:, :].rearrange("d (q e) -> d q e", q=3))
            convw_sb = consts.tile([D, DOUT], bf16)
            nc.sync.dma_start(convw_sb[:, :], conv_w3[:, :])
            convb_sb = consts.tile([DOUT, 1], f32)
            nc.sync.dma_start(convb_sb[:, :], conv_b[:].unsqueeze(1))
            linw_sb = consts.tile([DOUT, D], bf16)
            nc.sync.dma_start(linw_sb[:, :], lin_w[:, :])
            linb_sb = consts.tile([D, 1], f32)
            nc.sync.dma_start(linb_sb[:, :], lin_b[:].unsqueeze(1))
            rest_sb = consts.tile([D, CN], f32)
            nc.sync.dma_start(rest_sb[:, :], rest[:, :])
            mw_sb = consts.tile([P, D], bf16)
            nc.sync.dma_start(mw_sb[:, :], _bcast_ap(bass, mha_w[:], P))
            mb_sb = consts.tile([P, D], bf16)
            nc.sync.dma_start(mb_sb[:, :], _bcast_ap(bass, mha_b[:], P))
            nw_sb = consts.tile([P, D], f32)
            nc.sync.dma_start(nw_sb[:, :], _bcast_ap(bass, norm_w[:], P))
            nb_sb = consts.tile([P, D], f32)
            nc.sync.dma_start(nb_sb[:, :], _bcast_ap(bass, norm_b[:], P))

            # ---------------- constants ----------------
            idn = consts.tile([P, P], f32)
            masks.make_identity(nc, idn[:, :])
            idb = consts.tile([P, P], bf16)
            masks.make_identity(nc, idb[:, :])
            mneg2 = consts.tile([P, 1], f32)
            nc.gpsimd.memset(mneg2[:, :], -2.0)

            # persistent intermediates
            d_rows = {}   # (side, step) -> [128, 8, 64] bf16 rows of A^s x
            for side in ("q", "v"):
                for step in (1, 2):
                    d_rows[(side, step)] = big.tile(
                        [P, NT, D], bf16, tag=f"d{side}{step}", name=f"d{side}{step}")

            qrows = big.tile([P, LT, D], bf16)
            vrows = big.tile([P, LT, D], bf16)
            qcT8 = big.tile([D, L], fp8)              # qc^T (keys, fp8)
            vcT_bf = big.tile([D + 1, L], bf16)
            vr8 = big.tile([P, LT, P], fp8)           # (V|1) rows, heads at 32h
            qT8 = big.tile([D, H, CH], fp8)           # M_h qc_chunk^T (fp8)
            qcTc_bf = big.tile([D, CH], bf16)
            mv_q = big.tile([P, LT, 2], f32)
            mv_v = big.tile([P, LT, 2], f32)
            rs2 = big.tile([P, 2, LT], f32)           # rstd, dim1 = side
            oT_sb = big.tile([P, 2, 3, QT3], f32)     # attn out^T: (pair, third); head
                                                      # even at base 0, odd at base 64
            onr_sb = big.tile([P, 6, D], bf16)        # normalized attn out rows
            o2r_sb = big.tile([P, 6, D], f32)         # (o@wfc + qc) rows
            oln_sb = big.tile([P, 6, D], bf16)        # after mha_ln
            onT_sb = big.tile([D, 2, P], bf16)
            olnT_sb = big.tile([D, 3 * CN], bf16)
            zr = big.tile([P, 6, D], f32)
            mv2 = big.tile([P, 6, 2], f32)
            rst2 = big.tile([P, 6], f32)
            xT_sb = big.tile([D, CN], bf16)
            x1_sb = big.tile([DOUT, CN], bf16)
            x3T_sb = big.tile([D, CN], f32)
            xr_sb = big.tile([P, 2, D], f32)
            yout = big.tile([P, 2, D], f32)

            nc.gpsimd.memset(vcT_bf[D:D + 1, :], 1.0)   # ones row for denominators

            helper_rr = [0]   # round-robin counter for helper-engine work

            def hcopy(dst, src):
                """PSUM->SBUF casts: mostly DVE, every 3rd on ACT (Pool
                cannot read PSUM)."""
                helper_rr[0] += 1
                if helper_rr[0] % 3 == 0:
                    nc.scalar.copy(dst, src)
                else:
                    nc.vector.tensor_copy(dst, src)

            def rsqrt_newton(dst, src, shape, tag, iters=2, eng=None):
                """dst = 1/sqrt(src) via fast-inverse-sqrt + Newton (all on eng)."""
                e = eng or nc.vector
                hv = ntmp.tile(shape, f32, tag=tag + "h", name=tag + "h")
                e.tensor_scalar_mul(hv[:, :], src, 0.5)
                y = dst
                e.tensor_scalar(
                    out=y.bitcast(i32), in0=src.bitcast(i32),
                    scalar1=1, scalar2=None, op0=OP.logical_shift_right)
                e.tensor_scalar(
                    out=y.bitcast(i32), in0=y.bitcast(i32),
                    scalar1=-1, scalar2=None, op0=OP.bitwise_xor)
                e.tensor_scalar(
                    out=y.bitcast(i32), in0=y.bitcast(i32),
                    scalar1=RSQRT_MAGIC + 1, scalar2=None, op0=OP.add)
                t = ntmp.tile(shape, f32, tag=tag + "t", name=tag + "t")
                for _ in range(iters):
                    e.tensor_mul(t[:, :], y, y)
                    e.tensor_tensor(out=t[:, :], in0=t[:, :], in1=hv[:, :], op=OP.mult)
                    e.tensor_scalar(
                        out=t[:, :], in0=t[:, :], scalar1=-1.0, scalar2=1.5,
                        op0=OP.mult, op1=OP.add)
                    e.tensor_mul(y, y, t[:, :])

            def ln_grp(grp, tpool):
                """LN stats+apply+transposes for kt tiles of group grp, both
                sides.  Stats/newton/applies run on Pool (SBUF-only, engine
                otherwise idle); q-side stats stay on DVE for parallelism."""
                kts = list(range(grp * NT, (grp + 1) * NT))
                for srcs, mv in ((src_q, mv_q), (src_v, mv_v)):
                    for i in kts:
                        st = tmp.tile([P, 6], f32, tag="bnst")
                        nc.vector.bn_stats(st[:, :], srcs[i // NT][:, i % NT, :])
                        nc.vector.bn_aggr(mv[:, i, :], st[:, :])
                i0 = kts[0]
                ve = tmp.tile([P, 2, NT], f32, tag="ve")
                nc.vector.tensor_scalar_add(ve[:, 0, :], mv_q[:, i0:i0 + NT, 1], 1e-5)
                nc.vector.tensor_scalar_add(ve[:, 1, :], mv_v[:, i0:i0 + NT, 1], 1e-5)
                rsqrt_newton(rs2[:, :, i0:i0 + NT], ve[:, :, :], [P, 2, NT], "lng")
                for side_i, (srcs, rows, mv) in enumerate(
                        ((src_q, qrows, mv_q), (src_v, vrows, mv_v))):
                    for i in kts:
                        nc.vector.tensor_scalar(
                            out=rows[:, i, :], in0=srcs[i // NT][:, i % NT, :],
                            scalar1=mv[:, i, 0:1], scalar2=rs2[:, side_i, i:i + 1],
                            op0=OP.subtract, op1=OP.mult)
                # transposes + fp8/bf16 column copies
                for q0 in range(kts[0], kts[0] + NT, 4):
                    tpb = tpool.tile([D, 4, P], bf16, tag="tp", name="tpq")
                    for m in range(4):
                        nc.tensor.transpose(tpb[:, m, :], qrows[:, q0 + m, :], idb[:, :])
                    hcopy(qcT8[:, P * q0:P * (q0 + 4)],
                          tpb[:, :, :].rearrange("d m p -> d (m p)"))
                for q0 in range(kts[0], kts[0] + NT, 4):
                    tpb = tpool.tile([D, 4, P], bf16, tag="tp", name="tpv")
                    for m in range(4):
                        nc.tensor.transpose(tpb[:, m, :], vrows[:, q0 + m, :], idb[:, :])
                    nc.vector.tensor_copy(
                        vcT_bf[0:D, P * q0:P * (q0 + 4)],
                        tpb[:, :, :].rearrange("d m p -> d (m p)"))
                # V rows (heads padded to 32 lanes, ones col at 32h+16)
                for q0 in range(kts[0], kts[0] + NT, 4):
                    vps = tpool.tile([P, 4, P], f32, tag="tp", name="vps")
                    for m in range(4):
                        nc.tensor.matmul(vps[:, m, :],
                                         lhsT=vcT_bf[:, P * (q0 + m):P * (q0 + m + 1)],
                                         rhs=wv_sb[:, :], start=True, stop=True)
                    hcopy(vr8[:, q0:q0 + 4, :], vps[:, :, :])

            # ---- attention inner iteration ----
            exp_sched = [0]
            prev_ex = [None]
            av_state = {}

            def av_emit(pair, heads, expair):
                avs = av_state["avs"]
                for h in heads:
                    nc.tensor.matmul(
                        avs[h][:, :],
                        lhsT=vr8[:, 2 * pair:2 * pair + 2, 32 * h:32 * h + 32],
                        rhs=expair[:, :, h, :],
                        start=(pair == 0), stop=(pair == LT // 2 - 1),
                        perf_mode=PM.DoubleRow, skip_group_check=True)

            def attn_iter(t3, kt, scp, expair):
                # scores per head-pair (one PSUM bank each) so exp can run
                # at 512-col granularity on alternating engines
                for pp in range(2):
                    sc = scp.tile([P, 2, QT3], f32, tag=f"sc{pp}")
                    nc.tensor.matmul(
                        sc[:, :, :],
                        lhsT=qcT8[:, P * kt:P * (kt + 1)],
                        rhs=qT8[:, 2 * pp:2 * pp + 2, QT3 * t3:QT3 * (t3 + 1)],
                        start=True, stop=True)
                    exd = expair[:, kt % 2, 2 * pp:2 * pp + 2, :]
                    c = exp_sched[0]
                    exp_sched[0] += 1
                    if c % 3 < 2 or c >= 128:
                        nc.scalar.activation(exd, sc[:, :, :], AF.Exp,
                                             bias=mneg2[:, :], scale=1.0)
                    else:
                        nc.vector.tensor_scalar(
                            out=exd.bitcast(u8), in0=sc[:, :, :],
                            scalar1=C1_8, scalar2=C2_8, op0=OP.mult, op1=OP.add)
                # spread the pair's 4 attn@V matmuls across two kt slots so
                # the PE has filler work while exp(kt) completes (keeps the
                # in-order PE from stalling and dropping out of max p-state)
                if kt % 2 == 1:
                    av_emit(kt // 2, (0, 1), expair)
                    if kt == LT - 1:
                        av_emit(kt // 2, (2, 3), expair)
                elif kt > 0:
                    av_emit(kt // 2 - 1, (2, 3), prev_ex[0])
                prev_ex[0] = expair

            def o_chain(t3, tpool):
                """Normalize o~ by softmax denominators, apply w_fc + residual +
                mha_ln, build olnT columns for the pooling matmuls."""
                for h in range(H):
                    pair, b = h // 2, 64 * (h % 2)
                    for jj in range(2):
                        j = 2 * t3 + jj
                        tpo = tpool.tile([P, DV1], f32, tag="tp", name="tpo")
                        nc.tensor.transpose(
                            tpo[:, :],
                            oT_sb[b:b + DV1, pair, t3, P * jj:P * (jj + 1)],
                            idn[b:b + DV1, b:b + DV1],
                            tile_position=(b, 0))
                        rec = tmp.tile([P, 1], f32, tag="rec")
                        nc.vector.reciprocal(rec[:, :], tpo[:, DV:DV1])
                        nc.vector.tensor_scalar_mul(
                            onr_sb[:, j, DV * h:DV * (h + 1)], tpo[:, 0:DV], rec[:, :])
                for jj in range(2):
                    j = 2 * t3 + jj
                    tpn = tpool.tile([D, P], bf16, tag="tp", name="tpn")
                    nc.tensor.transpose(tpn[:, :], onr_sb[:, j, :], idb[:, :])
                    nc.vector.tensor_copy(onT_sb[:, jj, :], tpn[:, :])
                    o2p = tpool.tile([P, D], f32, tag="tp", name="o2p")
                    nc.tensor.matmul(o2p[:, :], lhsT=onT_sb[:, jj, :],
                                     rhs=wfc_sb[:, :], start=True, stop=True)
                    nc.vector.tensor_add(o2r_sb[:, j, :], o2p[:, :], zr[:, j, :])
                    st2 = tmp.tile([P, 6], f32, tag="bnst2")
                    nc.vector.bn_stats(st2[:, :], o2r_sb[:, j, :])
                    nc.vector.bn_aggr(mv2[:, j, :], st2[:, :])
                ve2 = tmp.tile([P, 2], f32, tag="ve2")
                nc.vector.tensor_scalar_add(ve2[:, :], mv2[:, 2 * t3:2 * t3 + 2, 1], 1e-6)
                rsqrt_newton(rst2[:, 2 * t3:2 * t3 + 2], ve2[:, :], [P, 2], "ml")
                for jj in range(2):
                    j = 2 * t3 + jj
                    nc.vector.tensor_scalar(
                        out=oln_sb[:, j, :], in0=o2r_sb[:, j, :],
                        scalar1=mv2[:, j, 0:1], scalar2=rst2[:, j:j + 1],
                        op0=OP.subtract, op1=OP.mult)
                    nc.gpsimd.tensor_mul(oln_sb[:, j, :], oln_sb[:, j, :], mw_sb[:, :])
                    nc.gpsimd.tensor_add(oln_sb[:, j, :], oln_sb[:, j, :], mb_sb[:, :])
                    tpl = tpool.tile([D, P], bf16, tag="tp", name="tpl")
                    nc.tensor.transpose(tpl[:, :], oln_sb[:, j, :], idb[:, :])
                    nc.vector.tensor_copy(olnT_sb[:, P * j:P * (j + 1)], tpl[:, :])

            src_q = [xqb_sb, d_rows[("q", 1)], d_rows[("q", 2)]]
            src_v = [xvb_sb, d_rows[("v", 1)], d_rows[("v", 2)]]

            with (
                tc.tile_pool(name="tp", bufs=2, space="PSUM") as tp_pool,
            ):
                # ===== chunk path: z = A_chunk x (feature-major), LN row-wise =====
                with tc.tile_pool(name="chk", bufs=1, space="PSUM") as chk:
                    zps = chk.tile([D, CH], f32, tag="zps")
                    for t in range(NT):
                        nc.tensor.matmul(zps[:, 0:512], lhsT=xqb_sb[:, t, :],
                                         rhs=acq_sb[:, t, 0:512], start=(t == 0), stop=(t == NT - 1))
                        nc.tensor.matmul(zps[:, 512:CH], lhsT=xqb_sb[:, t, :],
                                         rhs=acq_sb[:, t, 512:CH], start=(t == 0), stop=(t == NT - 1))
                    zT = tmp.tile([D, CH], bf16, tag="zT")
                    nc.vector.tensor_copy(zT[:, :], zps[:, :])
                    for j0, cnt in ((0, 4), (4, 2)):
                        tq = tp_pool.tile([P, 4, D], bf16, tag="tp", name="ztq")
                        for m in range(cnt):
                            nc.tensor.transpose(tq[:, m, :], zT[:, P * (j0 + m):P * (j0 + m + 1)],
                                                idb[:D, :D])
                        nc.vector.tensor_copy(zr[:, j0:j0 + cnt, :], tq[:, 0:cnt, :])
                    mvc = tmp.tile([P, 6, 2], f32, tag="mvc")
                    for j in range(6):
                        stc = tmp.tile([P, 6], f32, tag="bnst")
                        nc.vector.bn_stats(stc[:, :], zr[:, j, :])
                        nc.vector.bn_aggr(mvc[:, j, :], stc[:, :])
                    vec = tmp.tile([P, 6], f32, tag="vec")
                    nc.vector.tensor_scalar_add(vec[:, :], mvc[:, :, 1], 1e-5)
                    rsc = tmp.tile([P, 6], f32, tag="rsc")
                    rsqrt_newton(rsc[:, :], vec[:, :], [P, 6], "chk")
                    zq = tmp.tile([P, 6, D], bf16, tag="zq")
                    for j in range(6):
                        nc.vector.tensor_scalar(
                            out=zr[:, j, :], in0=zr[:, j, :],
                            scalar1=mvc[:, j, 0:1], scalar2=rsc[:, j:j + 1],
                            op0=OP.subtract, op1=OP.mult)
                        nc.vector.tensor_copy(zq[:, j, :], zr[:, j, :])
                    for j0, cnt in ((0, 4), (4, 2)):
                        tq2 = tp_pool.tile([D, 4, P], bf16, tag="tp", name="ztq2")
                        for m in range(cnt):
                            nc.tensor.transpose(tq2[:, m, :], zq[:, j0 + m, :], idb[:, :])
                        nc.vector.tensor_copy(
                            qcTc_bf[:, P * j0:P * (j0 + cnt)],
                            tq2[:, 0:cnt, :].rearrange("d m p -> d (m p)"))
                    # q~ per head pair: rows 0-63 = head 2p, 64-127 = head 2p+1
                    for pp in range(2):
                        qps = chk.tile([P, CH], f32, tag="sb", name="qps")
                        nc.tensor.matmul(qps[:, 0:512], lhsT=m_sb[:, P * pp:P * (pp + 1)],
                                         rhs=qcTc_bf[:, 0:512], start=True, stop=True)
                        nc.tensor.matmul(qps[:, 512:CH], lhsT=m_sb[:, P * pp:P * (pp + 1)],
                                         rhs=qcTc_bf[:, 512:CH], start=True, stop=True)
                        hcopy(qT8[:, 2 * pp, :], qps[0:D, :])
                        hcopy(qT8[:, 2 * pp + 1, :], qps[D:P, :])

                # ===== step-0 layernorm (kt 0..7 both sides) + V rows =====
                ln_grp(0, tp_pool)

                # ===== diffusion task list, interleaved with attention =====
                def diffuse_tile(at_sb, lhs_src, dst, i):
                    dps = tp_pool.tile([P, D], f32, tag="tp", name="dps")
                    for j in range(NT):
                        nc.tensor.matmul(
                            dps[:, :], lhsT=at_sb[:, j, P * i:P * (i + 1)],
                            rhs=lhs_src[:, j, :],
                            start=(j == 0), stop=(j == NT - 1))
                    hcopy(dst[:, i, :], dps[:, :])

                diff_tasks = []
                for at_sb_, lhs_, dst_ in (
                    (atq_sb, xqb_sb, d_rows[("q", 1)]),
                    (atv_sb, xvb_sb, d_rows[("v", 1)]),
                    (atq_sb, d_rows[("q", 1)], d_rows[("q", 2)]),
                    (atv_sb, d_rows[("v", 1)], d_rows[("v", 2)]),
                ):
                    for i_ in range(NT):
                        diff_tasks.append((at_sb_, lhs_, dst_, i_))
                diff_tasks = diff_tasks[::-1]  # pop from the end

                def emit_diff(n):
                    for _ in range(n):
                        if diff_tasks:
                            diffuse_tile(*diff_tasks.pop())

                with (
                    tc.tile_pool(name="psE", bufs=1, space="PSUM") as psE,
                    tc.tile_pool(name="psEa", bufs=1, space="PSUM") as psEa,
                    tc.tile_pool(name="expp", bufs=3) as expp,
                ):
                    def av_tiles(t3):
                        return [psEa.tile([32, QT3], f32, tag=f"avP{h}", name=f"av{t3}{h}")
                                for h in range(H)]

                    def flush(t3, avs):
                        for h in range(H):
                            nc.vector.tensor_copy(
                                oT_sb[64 * (h % 2):64 * (h % 2) + 32, h // 2, t3, :],
                                avs[h][:, :])

                    avs = av_tiles(0)
                    av_state["avs"] = avs
                    expair = None
                    for grp in range(3):
                        if grp > 0:
                            ln_grp(grp, tp_pool)
                        for kt in range(grp * NT, (grp + 1) * NT):
                            if kt % 2 == 0:
                                expair = expp.tile([P, 2, H, QT3], fp8, tag="ex")
                            attn_iter(0, kt, psE, expair)
                            emit_diff(2)
                    emit_diff(32)
                    flush(0, avs)
                    o_chain(0, tp_pool)
                    for t3 in (1, 2):
                        avs = av_tiles(t3)
                        av_state["avs"] = avs
                        for kt in range(LT):
                            if kt % 2 == 0:
                                expair = expp.tile([P, 2, H, QT3], fp8, tag="ex")
                            attn_iter(t3, kt, psE, expair)
                        flush(t3, avs)
                        if t3 == 1:
                            o_chain(1, tp_pool)

            # ================= epilogue =================
            with (
                tc.tile_pool(name="psF", bufs=4, space="PSUM") as psF,
                tc.tile_pool(name="psFf", bufs=1, space="PSUM") as psFf,
            ):
                o_chain(2, psF)
                # pooling: xT[d, n] = sum_q Pq^T olnT[:, 3n+q]
                xps = psFf.tile([D, CN], f32, tag="xps")
                olv = olnT_sb[:, :].rearrange("d (j s) -> d s j", s=3)
                for q in range(3):
                    nc.tensor.matmul(xps[:, :], lhsT=pq_sb[:, q, :], rhs=olv[:, q, :],
                                     start=(q == 0), stop=(q == 2))
                nc.vector.tensor_copy(xT_sb[:, :], xps[:, :])
                # conv/relu/lin/residual (1/3 pool-mean folded into conv_w3)
                x1ps = psFf.tile([DOUT, CN], f32, tag="x1ps")
                nc.tensor.matmul(x1ps[:, :], lhsT=convw_sb[:, :], rhs=xT_sb[:, :],
                                 start=True, stop=True)
                nc.scalar.activation(x1_sb[:, :], x1ps[:, :], AF.Relu,
                                     bias=convb_sb[:, :], scale=1.0)
                x2ps = psFf.tile([D, CN], f32, tag="x2ps")
                nc.tensor.matmul(x2ps[:, :], lhsT=linw_sb[:, :], rhs=x1_sb[:, :],
                                 start=True, stop=True)
                nc.vector.tensor_scalar_add(x3T_sb[:, :], x2ps[:, :], linb_sb[:, :])
                nc.vector.tensor_add(x3T_sb[:, :], x3T_sb[:, :], rest_sb[:, :])
                # rows + final LN (affine, eps 1e-5)
                for n2 in range(2):
                    tpf = psF.tile([P, D], f32, tag="tp")
                    nc.tensor.transpose(tpf[:, :], x3T_sb[:, P * n2:P * (n2 + 1)], idn[:D, :D])
                    nc.vector.tensor_copy(xr_sb[:, n2, :], tpf[:, :])
                mv3 = tmp.tile([P, 2, 2], f32, tag="mv3")
                for n2 in range(2):
                    st3 = tmp.tile([P, 6], f32, tag="bnst3")
                    nc.vector.bn_stats(st3[:, :], xr_sb[:, n2, :])
                    nc.vector.bn_aggr(mv3[:, n2, :], st3[:, :])
                ve3 = tmp.tile([P, 2], f32, tag="ve3")
                nc.vector.tensor_scalar_add(ve3[:, :], mv3[:, :, 1], 1e-5)
                rst3 = tmp.tile([P, 2], f32, tag="rst3")
                rsqrt_newton(rst3[:, :], ve3[:, :], [P, 2], "fl")
                for n2 in range(2):
                    nc.vector.tensor_scalar(
                        out=yout[:, n2, :], in0=xr_sb[:, n2, :],
                        scalar1=mv3[:, n2, 0:1], scalar2=rst3[:, n2:n2 + 1],
                        op0=OP.subtract, op1=OP.mult)
                    nc.gpsimd.tensor_mul(yout[:, n2, :], yout[:, n2, :], nw_sb[:, :])
                    nc.gpsimd.tensor_add(yout[:, n2, :], yout[:, n2, :], nb_sb[:, :])
                nc.sync.dma_start(out_d[:, :].rearrange("(t p) d -> p t d", p=P), yout[:, :, :])

    nc.finalize()
    return nc


def _prep_in_maps(inputs):
    import ml_dtypes
    bf = ml_dtypes.bfloat16

    q_x = np.asarray(inputs["q_x"], np.float32)
    v_x = np.asarray(inputs["v_x"], np.float32)
    q_adj = np.asarray(inputs["q_adj"], np.float32)
    v_adj = np.asarray(inputs["v_adj"], np.float32)
    w_qs = np.asarray(inputs["w_qs"], np.float32)
    w_ks = np.asarray(inputs["w_ks"], np.float32)
    w_vs = np.asarray(inputs["w_vs"], np.float32)
    w_fc = np.asarray(inputs["w_fc"], np.float32)
    mha_ln_w = np.asarray(inputs["mha_ln_w"], np.float32)
    mha_ln_b = np.asarray(inputs["mha_ln_b"], np.float32)
    conv_w = np.asarray(inputs["conv_w"], np.float32)
    conv_b = np.asarray(inputs["conv_b"], np.float32)
    lin_w = np.asarray(inputs["lin_w"], np.float32)
    lin_b = np.asarray(inputs["lin_b"], np.float32)
    norm_w = np.asarray(inputs["norm_w"], np.float32)
    norm_b = np.asarray(inputs["norm_b"], np.float32)

    # M_h = (Wq_h @ Wk_h^T) / sqrt(DK), head pairs side by side
    m_pair = np.zeros((D, 2 * P), np.float32)
    for h in range(H):
        m_pair[:, D * h:D * (h + 1)] = (
            w_qs[:, DK * h:DK * (h + 1)] @ w_ks[:, DK * h:DK * (h + 1)].T
        ) / np.sqrt(DK)
    # V projection: head h at cols 32h..32h+16 (16 V cols + ones col)
    wv32 = np.zeros((D + 1, P), np.float32)
    for h in range(H):
        wv32[:D, 32 * h:32 * h + DV] = w_vs[:, DV * h:DV * (h + 1)]
        wv32[D, 32 * h + DV] = 1.0
    # pooling selectors: out(n,d) = sum_q sum_c Pq[c,d] oln[3n+q, c]
    pq_w = np.zeros((D, 3, D), np.float32)
    for d in range(D):
        for s in range(STEPS):
            q, c = divmod(3 * d + s, D)
            pq_w[c, q, d] = 1.0
    conv_w3 = conv_w / 3.0

    # column permutation for A^T: col' i*128+m holds original node m*8+i
    idx = np.arange(N)
    colperm = (idx % P) * NT + idx // P

    shared = dict(
        m_pair=m_pair.astype(bf),
        wv32=wv32.astype(bf),
        wfc=w_fc.astype(bf),
        mha_w=mha_ln_w.astype(bf), mha_b=mha_ln_b.astype(bf),
        pq_w=pq_w.reshape(D, 3 * D).astype(bf),
        conv_w3=conv_w3.astype(bf), conv_b=conv_b,
        lin_w=lin_w.astype(bf), lin_b=lin_b,
        norm_w=norm_w, norm_b=norm_b,
    )

    per_batch = []
    for b in range(B):
        A, Av = q_adj[b], v_adj[b]
        A2 = A @ A
        G = np.concatenate([np.eye(N, dtype=np.float32), A, A2], axis=0)  # [3N, N]
        per_batch.append(dict(
            xqb=q_x[b].astype(bf), xvb=v_x[b].astype(bf),
            atq=np.ascontiguousarray(A.T[:, colperm]).astype(bf),
            atv=np.ascontiguousarray(Av.T[:, colperm]).astype(bf),
            G=G,
        ))

    in_maps = []
    for c in range(8):
        b, g = c // 4, c % 4
        pb = per_batch[b]
        acq = np.ascontiguousarray(pb["G"][CH * g:CH * (g + 1)].T).astype(bf)  # [N, CH]
        rest = np.ascontiguousarray(q_x[b, CN * g:CN * (g + 1)].T)             # [D, CN]
        m = dict(shared)
        m.update(xqb=pb["xqb"], xvb=pb["xvb"],
                 atq=pb["atq"], atv=pb["atv"], acq=acq, rest=rest)
        in_maps.append(m)
    return in_maps


def _run(inputs, trace=False, **kw):
    from concourse.bass_utils import run_bass_kernel_spmd

    if "nc" not in _CACHE:
        _CACHE["nc"] = _build_nc()
    nc = _CACHE["nc"]
    in_maps = _prep_in_maps(inputs)
    res = run_bass_kernel_spmd(nc, in_maps, core_ids=list(range(8)), trace=trace, **kw)
    out = np.empty((B, N, D), np.float32)
    for c in range(8):
        b, g = c // 4, c % 4
        out[b, CN * g:CN * (g + 1)] = res.results[c]["out"]
    return out, res


def kernel(**inputs) -> np.ndarray:
    out, _ = _run(inputs, trace=False)
    return out


# revision 27
# speedup vs baseline: 1.1650x; 1.0052x over previous
"""Trainium2 Bass kernel for nn_DiffAtten (diffusion GNN + multi-head attention).

Model (per batch b): qc = LN([x; Ax; A^2x]) (L=3072 rows), vc likewise with v-graph;
MHA over L with H=4 heads of dim 16; o = attn-out @ w_fc + qc; LN; pool triples of
rows; conv+relu+linear+residual; final LN.  Output [2, 1024, 64] f32.

Sharding: 8 cores = 2 batches x 4 groups.  Core (b, g) computes attention for the
L-contiguous query chunk [768g, 768(g+1)) (covering output nodes [256g, 256(g+1))
after triple-pooling) against the full 3072-key side, recomputed on-core.

Numerics/engine strategy (validated against the f32 reference, ~4e-4 rel err;
the PE on this instance is activity-throttled to ~0.5 util, so the design
minimizes PE work and keeps the in-order PE queue stall-free):
  - scores contract M_h-folded queries against qc^T, both fp8e4 (64-deep,
    f32 PSUM accumulation); attn@V runs as fp8 DoubleRow matmuls (two key
    tiles per pass, heads padded to 32-lane blocks with a ones column so
    softmax denominators fall out of the same accumulation).  The four
    per-pair attn@V matmuls are spread over two kt slots as PE filler.
  - exp runs with bias -2 (keeps e^s inside fp8e4 range), split ACT 2/3 :
    DVE 1/3 (alternating evenly in the tail): ACT uses the real activation
    table writing fp8 directly; DVE computes Schraudolph bit-space exp in
    one tensor_scalar (u8 = round(s*c1+c2) IS the fp8 bit pattern; the
    f32->u8 convert saturates, putting the negative tail exactly on +0.0).
  - everything else is bf16 (rows, transposes, weights) so DVE hits its
    2x 16-bit mode and PE transposes run 1 cycle/row; LN stats/newton-rsqrt
    on DVE, affine tensor_tensor ops on the otherwise idle Pool engine
    (Pool cannot read PSUM or run pointer-scalar tensor_scalar ops).
  - inputs stream in p-outer layout (node = p*8 + t) so every DMA is 128
    contiguous per-partition descriptors; A^T columns are host-permuted to
    match.  atq issues from the ACT queue so both adjacencies transfer from
    t=0; acq is split in two so the chunk path starts ~3us in.
  - the pool->conv->linear epilogue stays in SBUF: triple-row pooling is
    three stride-3 PE matmuls against 0/1 selection matrices (1/3 folded
    into conv_w), so nothing round-trips through DRAM.
"""

import numpy as np

B, N, D = 2, 1024, 64
H, DK, DV = 4, 16, 16
DOUT = 128
STEPS = 3
L = STEPS * N          # 3072
P = 128
NT = N // P            # 8 node tiles
LT = L // P            # 24 L tiles
CH = L // 4            # 768 q-chunk per core
CN = N // 4            # 256 output nodes per core
QT3 = CH // 3          # 256 q columns per third
DV1 = DV + 1           # 17
RSQRT_MAGIC = 0x5F3759DF
C1_8 = 8.0 / np.log(2.0)           # fp8e4m3 Schraudolph slope
C2_8 = 7.0 * 8.0 - 2.0 * C1_8     # bias for exp(s - 2)

_CACHE = {}


def _bcast_ap(bass_mod, ap, parts):
    """[F] dram AP -> [parts, F] broadcast AP (partition step 0)."""
    return bass_mod.AP(tensor=ap.tensor, offset=ap.offset, ap=[[0, parts]] + list(ap.ap))


def _build_nc():
    import concourse.bass as bass
    import concourse.bacc as bacc
    import concourse.tile as tile
    from concourse import mybir, masks

    f32 = mybir.dt.float32
    i32 = mybir.dt.int32
    u8 = mybir.dt.uint8
    bf16 = mybir.dt.bfloat16
    fp8 = mybir.dt.float8e4
    AF = mybir.ActivationFunctionType
    OP = mybir.AluOpType
    PM = mybir.MatmulPerfMode

    nc = bacc.Bacc(None, target_bir_lowering=False)

    # ---- kernel I/O (per-core slices supplied by the host) ----
    xqb = nc.dram_tensor("xqb", [N, D], bf16, kind="ExternalInput")
    xvb = nc.dram_tensor("xvb", [N, D], bf16, kind="ExternalInput")
    atq = nc.dram_tensor("atq", [N, N], bf16, kind="ExternalInput")   # A^T, cols permuted
    atv = nc.dram_tensor("atv", [N, N], bf16, kind="ExternalInput")
    acq = nc.dram_tensor("acq", [N, CH], bf16, kind="ExternalInput")  # chunk operator^T
    m_pair = nc.dram_tensor("m_pair", [D, 2 * P], bf16, kind="ExternalInput")
    wv32 = nc.dram_tensor("wv32", [D + 1, P], bf16, kind="ExternalInput")
    wfc = nc.dram_tensor("wfc", [D, D], bf16, kind="ExternalInput")
    mha_w = nc.dram_tensor("mha_w", [D], bf16, kind="ExternalInput")
    mha_b = nc.dram_tensor("mha_b", [D], bf16, kind="ExternalInput")
    pq_w = nc.dram_tensor("pq_w", [D, 3 * D], bf16, kind="ExternalInput")
    conv_w3 = nc.dram_tensor("conv_w3", [D, DOUT], bf16, kind="ExternalInput")
    conv_b = nc.dram_tensor("conv_b", [DOUT], f32, kind="ExternalInput")
    lin_w = nc.dram_tensor("lin_w", [DOUT, D], bf16, kind="ExternalInput")
    lin_b = nc.dram_tensor("lin_b", [D], f32, kind="ExternalInput")
    norm_w = nc.dram_tensor("norm_w", [D], f32, kind="ExternalInput")
    norm_b = nc.dram_tensor("norm_b", [D], f32, kind="ExternalInput")
    rest = nc.dram_tensor("rest", [D, CN], f32, kind="ExternalInput")
    out_d = nc.dram_tensor("out", [CN, D], f32, kind="ExternalOutput")

    with tile.TileContext(nc) as tc:
        with (
            tc.tile_pool(name="consts", bufs=1) as consts,
            tc.tile_pool(name="big", bufs=1) as big,
            tc.tile_pool(name="tmp", bufs=4) as tmp,
            tc.tile_pool(name="ntmp", bufs=2) as ntmp,
        ):
            # ---------------- input DMAs ----------------
            # Big adjacency loads issue from ACT/DVE queues so their
            # transfers start immediately; SP issues the rest in
            # dependency order (chunk path first).
            xqb_sb = big.tile([P, NT, D], bf16)
            nc.sync.dma_start(xqb_sb[:, :, :], xqb[:, :].rearrange("(p t) d -> p t d", p=P))
            acq_sb = big.tile([P, NT, CH], bf16)
            acq_v = acq[:, :].rearrange("(p t) c -> p t c", p=P)
            nc.sync.dma_start(acq_sb[:, 0:4, :], acq_v[:, 0:4, :])
            nc.sync.dma_start(acq_sb[:, 4:NT, :], acq_v[:, 4:NT, :])
            atq_sb = big.tile([P, NT, N], bf16)
            nc.scalar.dma_start(atq_sb[:, :, :], atq[:, :].rearrange("(p t) i -> p t i", p=P))
            atv_sb = big.tile([P, NT, N], bf16)
            nc.sync.dma_start(atv_sb[:, :, :], atv[:, :].rearrange("(p t) i -> p t i", p=P))
            xvb_sb = big.tile([P, NT, D], bf16)
            nc.sync.dma_start(xvb_sb[:, :, :], xvb[:, :].rearrange("(p t) d -> p t d", p=P))
            m_sb = consts.tile([D, 2 * P], bf16)
            nc.sync.dma_start(m_sb[:, :], m_pair[:, :])
            wv_sb = consts.tile([D + 1, P], bf16)
            nc.sync.dma_start(wv_sb[:, :], wv32[:, :])
            wfc_sb = consts.tile([D, D], bf16)
            nc.sync.dma_start(wfc_sb[:, :], wfc[:, :])
            pq_sb = consts.tile([D, 3, D], bf16)
            nc.sync.dma_start(pq_sb[:, :, :], pq_w[:, :].rearrange("d (q e) -> d q e", q=3))
            convw_sb = consts.tile([D, DOUT], bf16)
            nc.sync.dma_start(convw_sb[:, :], conv_w3[:, :])
            convb_sb = consts.tile([DOUT, 1], f32)
            nc.sync.dma_start(convb_sb[:, :], conv_b[:].unsqueeze(1))
            linw_sb = consts.tile([DOUT, D], bf16)
            nc.sync.dma_start(linw_sb[:, :], lin_w[:, :])
            linb_sb = consts.tile([D, 1], f32)
            nc.sync.dma_start(linb_sb[:, :], lin_b[:].unsqueeze(1))
            rest_sb = consts.tile([D, CN], f32)
            nc.sync.dma_start(rest_sb[:, :], rest[:, :])
            mw_sb = consts.tile([P, D], bf16)
            nc.sync.dma_start(mw_sb[:, :], _bcast_ap(bass, mha_w[:], P))
            mb_sb = consts.tile([P, D], bf16)
            nc.sync.dma_start(mb_sb[:, :], _bcast_ap(bass, mha_b[:], P))
            nw_sb = consts.tile([P, D], f32)
            nc.sync.dma_start(nw_sb[:, :], _bcast_ap(bass, norm_w[:], P))
            nb_sb = consts.tile([P, D], f32)
            nc.sync.dma_start(nb_sb[:, :], _bcast_ap(bass, norm_b[:], P))

            # ---------------- constants ----------------
            idn = consts.tile([P, P], f32)
            masks.make_identity(nc, idn[:, :])
            idb = consts.tile([P, P], bf16)
            masks.make_identity(nc, idb[:, :])
            mneg2 = consts.tile([P, 1], f32)
            nc.gpsimd.memset(mneg2[:, :], -2.0)

            # persistent intermediates
            d_rows = {}   # (side, step) -> [128, 8, 64] bf16 rows of A^s x
            for side in ("q", "v"):
                for step in (1, 2):
                    d_rows[(side, step)] = big.tile(
                        [P, NT, D], bf16, tag=f"d{side}{step}", name=f"d{side}{step}")

            qrows = big.tile([P, LT, D], bf16)
            vrows = big.tile([P, LT, D], bf16)
            qcT8 = big.tile([D, L], fp8)              # qc^T (keys, fp8)
            vcT_bf = big.tile([D + 1, L], bf16)
            vr8 = big.tile([P, LT, P], fp8)           # (V|1) rows, heads at 32h
            qT8 = big.tile([D, H, CH], fp8)           # M_h qc_chunk^T (fp8)
            qcTc_bf = big.tile([D, CH], bf16)
            mv_q = big.tile([P, LT, 2], f32)
            mv_v = big.tile([P, LT, 2], f32)
            rs2 = big.tile([P, 2, LT], f32)           # rstd, dim1 = side
            oT_sb = big.tile([P, 2, 3, QT3], f32)     # attn out^T: (pair, third); head
                                                      # even at base 0, odd at base 64
            onr_sb = big.tile([P, 6, D], bf16)        # normalized attn out rows
            o2r_sb = big.tile([P, 6, D], f32)         # (o@wfc + qc) rows
            oln_sb = big.tile([P, 6, D], bf16)        # after mha_ln
            onT_sb = big.tile([D, 2, P], bf16)
            olnT_sb = big.tile([D, 3 * CN], bf16)
            zr = big.tile([P, 6, D], f32)
            mv2 = big.tile([P, 6, 2], f32)
            rst2 = big.tile([P, 6], f32)
            xT_sb = big.tile([D, CN], bf16)
            x1_sb = big.tile([DOUT, CN], bf16)
            x3T_sb = big.tile([D, CN], f32)
            xr_sb = big.tile([P, 2, D], f32)
            yout = big.tile([P, 2, D], f32)

            nc.gpsimd.memset(vcT_bf[D:D + 1, :], 1.0)   # ones row for denominators

            helper_rr = [0]   # round-robin counter for helper-engine work

            def hcopy(dst, src):
                """PSUM->SBUF casts: mostly DVE, every 3rd on ACT (Pool
                cannot read PSUM)."""
                helper_rr[0] += 1
                if helper_rr[0] % 2 == 0:
                    nc.scalar.copy(dst, src)
                else:
                    nc.vector.tensor_copy(dst, src)

            def rsqrt_newton(dst, src, shape, tag, iters=2, eng=None):
                """dst = 1/sqrt(src) via fast-inverse-sqrt + Newton (all on eng)."""
                e = eng or nc.vector
                hv = ntmp.tile(shape, f32, tag=tag + "h", name=tag + "h")
                e.tensor_scalar_mul(hv[:, :], src, 0.5)
                y = dst
                e.tensor_scalar(
                    out=y.bitcast(i32), in0=src.bitcast(i32),
                    scalar1=1, scalar2=None, op0=OP.logical_shift_right)
                e.tensor_scalar(
                    out=y.bitcast(i32), in0=y.bitcast(i32),
                    scalar1=-1, scalar2=None, op0=OP.bitwise_xor)
                e.tensor_scalar(
                    out=y.bitcast(i32), in0=y.bitcast(i32),
                    scalar1=RSQRT_MAGIC + 1, scalar2=None, op0=OP.add)
                t = ntmp.tile(shape, f32, tag=tag + "t", name=tag + "t")
                for _ in range(iters):
                    e.tensor_mul(t[:, :], y, y)
                    e.tensor_tensor(out=t[:, :], in0=t[:, :], in1=hv[:, :], op=OP.mult)
                    e.tensor_scalar(
                        out=t[:, :], in0=t[:, :], scalar1=-1.0, scalar2=1.5,
                        op0=OP.mult, op1=OP.add)
                    e.tensor_mul(y, y, t[:, :])

            def ln_grp(grp, tpool):
                """LN stats+apply+transposes for kt tiles of group grp, both
                sides.  Stats/newton/applies run on Pool (SBUF-only, engine
                otherwise idle); q-side stats stay on DVE for parallelism."""
                kts = list(range(grp * NT, (grp + 1) * NT))
                for srcs, mv in ((src_q, mv_q), (src_v, mv_v)):
                    for i in kts:
                        st = tmp.tile([P, 6], f32, tag="bnst")
                        nc.vector.bn_stats(st[:, :], srcs[i // NT][:, i % NT, :])
                        nc.vector.bn_aggr(mv[:, i, :], st[:, :])
                i0 = kts[0]
                ve = tmp.tile([P, 2, NT], f32, tag="ve")
                nc.vector.tensor_scalar_add(ve[:, 0, :], mv_q[:, i0:i0 + NT, 1], 1e-5)
                nc.vector.tensor_scalar_add(ve[:, 1, :], mv_v[:, i0:i0 + NT, 1], 1e-5)
                rsqrt_newton(rs2[:, :, i0:i0 + NT], ve[:, :, :], [P, 2, NT], "lng")
                for side_i, (srcs, rows, mv) in enumerate(
                        ((src_q, qrows, mv_q), (src_v, vrows, mv_v))):
                    for i in kts:
                        nc.vector.tensor_scalar(
                            out=rows[:, i, :], in0=srcs[i // NT][:, i % NT, :],
                            scalar1=mv[:, i, 0:1], scalar2=rs2[:, side_i, i:i + 1],
                            op0=OP.subtract, op1=OP.mult)
                # transposes + fp8/bf16 column copies
                for q0 in range(kts[0], kts[0] + NT, 4):
                    tpb = tpool.tile([D, 4, P], bf16, tag="tp", name="tpq")
                    for m in range(4):
                        nc.tensor.transpose(tpb[:, m, :], qrows[:, q0 + m, :], idb[:, :])
                    hcopy(qcT8[:, P * q0:P * (q0 + 4)],
                          tpb[:, :, :].rearrange("d m p -> d (m p)"))
                for q0 in range(kts[0], kts[0] + NT, 4):
                    tpb = tpool.tile([D, 4, P], bf16, tag="tp", name="tpv")
                    for m in range(4):
                        nc.tensor.transpose(tpb[:, m, :], vrows[:, q0 + m, :], idb[:, :])
                    nc.vector.tensor_copy(
                        vcT_bf[0:D, P * q0:P * (q0 + 4)],
                        tpb[:, :, :].rearrange("d m p -> d (m p)"))
                # V rows (heads padded to 32 lanes, ones col at 32h+16)
                for q0 in range(kts[0], kts[0] + NT, 4):
                    vps = tpool.tile([P, 4, P], f32, tag="tp", name="vps")
                    for m in range(4):
                        nc.tensor.matmul(vps[:, m, :],
                                         lhsT=vcT_bf[:, P * (q0 + m):P * (q0 + m + 1)],
                                         rhs=wv_sb[:, :], start=True, stop=True)
                    hcopy(vr8[:, q0:q0 + 4, :], vps[:, :, :])

            # ---- attention inner iteration ----
            exp_sched = [0]
            prev_ex = [None]
            av_state = {}

            def av_emit(pair, heads, expair):
                avs = av_state["avs"]
                for h in heads:
                    nc.tensor.matmul(
                        avs[h][:, :],
                        lhsT=vr8[:, 2 * pair:2 * pair + 2, 32 * h:32 * h + 32],
                        rhs=expair[:, :, h, :],
                        start=(pair == 0), stop=(pair == LT // 2 - 1),
                        perf_mode=PM.DoubleRow, skip_group_check=True)

            def attn_iter(t3, kt, scp, expair):
                # scores per head-pair (one PSUM bank each) so exp can run
                # at 512-col granularity on alternating engines
                for pp in range(2):
                    sc = scp.tile([P, 2, QT3], f32, tag=f"sc{pp}")
                    nc.tensor.matmul(
                        sc[:, :, :],
                        lhsT=qcT8[:, P * kt:P * (kt + 1)],
                        rhs=qT8[:, 2 * pp:2 * pp + 2, QT3 * t3:QT3 * (t3 + 1)],
                        start=True, stop=True)
                    exd = expair[:, kt % 2, 2 * pp:2 * pp + 2, :]
                    c = exp_sched[0]
                    exp_sched[0] += 1
                    if (c % 3 < 2) if c < 128 else (c % 2 == 0):
                        nc.scalar.activation(exd, sc[:, :, :], AF.Exp,
                                             bias=mneg2[:, :], scale=1.0)
                    else:
                        nc.vector.tensor_scalar(
                            out=exd.bitcast(u8), in0=sc[:, :, :],
                            scalar1=C1_8, scalar2=C2_8, op0=OP.mult, op1=OP.add)
                # spread the pair's 4 attn@V matmuls across two kt slots so
                # the PE has filler work while exp(kt) completes (keeps the
                # in-order PE from stalling and dropping out of max p-state)
                if kt % 2 == 1:
                    av_emit(kt // 2, (0, 1), expair)
                    if kt == LT - 1:
                        av_emit(kt // 2, (2, 3), expair)
                elif kt > 0:
                    av_emit(kt // 2 - 1, (2, 3), prev_ex[0])
                prev_ex[0] = expair

            def o_chain(t3, tpool):
                """Normalize o~ by softmax denominators, apply w_fc + residual +
                mha_ln, build olnT columns for the pooling matmuls."""
                for h in range(H):
                    pair, b = h // 2, 64 * (h % 2)
                    for jj in range(2):
                        j = 2 * t3 + jj
                        tpo = tpool.tile([P, DV1], f32, tag="tp", name="tpo")
                        nc.tensor.transpose(
                            tpo[:, :],
                            oT_sb[b:b + DV1, pair, t3, P * jj:P * (jj + 1)],
                            idn[b:b + DV1, b:b + DV1],
                            tile_position=(b, 0))
                        rec = tmp.tile([P, 1], f32, tag="rec")
                        nc.vector.reciprocal(rec[:, :], tpo[:, DV:DV1])
                        nc.vector.tensor_scalar_mul(
                            onr_sb[:, j, DV * h:DV * (h + 1)], tpo[:, 0:DV], rec[:, :])
                for jj in range(2):
                    j = 2 * t3 + jj
                    tpn = tpool.tile([D, P], bf16, tag="tp", name="tpn")
                    nc.tensor.transpose(tpn[:, :], onr_sb[:, j, :], idb[:, :])
                    nc.vector.tensor_copy(onT_sb[:, jj, :], tpn[:, :])
                    o2p = tpool.tile([P, D], f32, tag="tp", name="o2p")
                    nc.tensor.matmul(o2p[:, :], lhsT=onT_sb[:, jj, :],
                                     rhs=wfc_sb[:, :], start=True, stop=True)
                    nc.vector.tensor_add(o2r_sb[:, j, :], o2p[:, :], zr[:, j, :])
                    st2 = tmp.tile([P, 6], f32, tag="bnst2")
                    nc.vector.bn_stats(st2[:, :], o2r_sb[:, j, :])
                    nc.vector.bn_aggr(mv2[:, j, :], st2[:, :])
                ve2 = tmp.tile([P, 2], f32, tag="ve2")
                nc.vector.tensor_scalar_add(ve2[:, :], mv2[:, 2 * t3:2 * t3 + 2, 1], 1e-6)
                rsqrt_newton(rst2[:, 2 * t3:2 * t3 + 2], ve2[:, :], [P, 2], "ml")
                for jj in range(2):
                    j = 2 * t3 + jj
                    nc.vector.tensor_scalar(
                        out=oln_sb[:, j, :], in0=o2r_sb[:, j, :],
                        scalar1=mv2[:, j, 0:1], scalar2=rst2[:, j:j + 1],
                        op0=OP.subtract, op1=OP.mult)
                    nc.gpsimd.tensor_mul(oln_sb[:, j, :], oln_sb[:, j, :], mw_sb[:, :])
                    nc.gpsimd.tensor_add(oln_sb[:, j, :], oln_sb[:, j, :], mb_sb[:, :])
                    tpl = tpool.tile([D, P], bf16, tag="tp", name="tpl")
                    nc.tensor.transpose(tpl[:, :], oln_sb[:, j, :], idb[:, :])
                    nc.vector.tensor_copy(olnT_sb[:, P * j:P * (j + 1)], tpl[:, :])

            src_q = [xqb_sb, d_rows[("q", 1)], d_rows[("q", 2)]]
            src_v = [xvb_sb, d_rows[("v", 1)], d_rows[("v", 2)]]

            with (
                tc.tile_pool(name="tp", bufs=2, space="PSUM") as tp_pool,
            ):
                # ===== chunk path: z = A_chunk x (feature-major), LN row-wise =====
                with tc.tile_pool(name="chk", bufs=1, space="PSUM") as chk:
                    zps = chk.tile([D, CH], f32, tag="zps")
                    for t in range(NT):
                        nc.tensor.matmul(zps[:, 0:512], lhsT=xqb_sb[:, t, :],
                                         rhs=acq_sb[:, t, 0:512], start=(t == 0), stop=(t == NT - 1))
                        nc.tensor.matmul(zps[:, 512:CH], lhsT=xqb_sb[:, t, :],
                                         rhs=acq_sb[:, t, 512:CH], start=(t == 0), stop=(t == NT - 1))
                    zT = tmp.tile([D, CH], bf16, tag="zT")
                    nc.vector.tensor_copy(zT[:, :], zps[:, :])
                    for j0, cnt in ((0, 4), (4, 2)):
                        tq = tp_pool.tile([P, 4, D], bf16, tag="tp", name="ztq")
                        for m in range(cnt):
                            nc.tensor.transpose(tq[:, m, :], zT[:, P * (j0 + m):P * (j0 + m + 1)],
                                                idb[:D, :D])
                        nc.vector.tensor_copy(zr[:, j0:j0 + cnt, :], tq[:, 0:cnt, :])
                    mvc = tmp.tile([P, 6, 2], f32, tag="mvc")
                    for j in range(6):
                        stc = tmp.tile([P, 6], f32, tag="bnst")
                        nc.vector.bn_stats(stc[:, :], zr[:, j, :])
                        nc.vector.bn_aggr(mvc[:, j, :], stc[:, :])
                    vec = tmp.tile([P, 6], f32, tag="vec")
                    nc.vector.tensor_scalar_add(vec[:, :], mvc[:, :, 1], 1e-5)
                    rsc = tmp.tile([P, 6], f32, tag="rsc")
                    rsqrt_newton(rsc[:, :], vec[:, :], [P, 6], "chk")
                    zq = tmp.tile([P, 6, D], bf16, tag="zq")
                    for j in range(6):
                        nc.vector.tensor_scalar(
                            out=zr[:, j, :], in0=zr[:, j, :],
                            scalar1=mvc[:, j, 0:1], scalar2=rsc[:, j:j + 1],
                            op0=OP.subtract, op1=OP.mult)
                        nc.vector.tensor_copy(zq[:, j, :], zr[:, j, :])
                    for j0, cnt in ((0, 4), (4, 2)):
                        tq2 = tp_pool.tile([D, 4, P], bf16, tag="tp", name="ztq2")
                        for m in range(cnt):
                            nc.tensor.transpose(tq2[:, m, :], zq[:, j0 + m, :], idb[:, :])
                        nc.vector.tensor_copy(
                            qcTc_bf[:, P * j0:P * (j0 + cnt)],
                            tq2[:, 0:cnt, :].rearrange("d m p -> d (m p)"))
                    # q~ per head pair: rows 0-63 = head 2p, 64-127 = head 2p+1
                    for pp in range(2):
                        qps = chk.tile([P, CH], f32, tag="sb", name="qps")
                        nc.tensor.matmul(qps[:, 0:512], lhsT=m_sb[:, P * pp:P * (pp + 1)],
                                         rhs=qcTc_bf[:, 0:512], start=True, stop=True)
                        nc.tensor.matmul(qps[:, 512:CH], lhsT=m_sb[:, P * pp:P * (pp + 1)],
                                         rhs=qcTc_bf[:, 512:CH], start=True, stop=True)
                        hcopy(qT8[:, 2 * pp, :], qps[0:D, :])
                        hcopy(qT8[:, 2 * pp + 1, :], qps[D:P, :])

                # ===== step-0 layernorm (kt 0..7 both sides) + V rows =====
                ln_grp(0, tp_pool)

                # ===== diffusion task list, interleaved with attention =====
                def diffuse_tile(at_sb, lhs_src, dst, i):
                    dps = tp_pool.tile([P, D], f32, tag="tp", name="dps")
                    for j in range(NT):
                        nc.tensor.matmul(
                            dps[:, :], lhsT=at_sb[:, j, P * i:P * (i + 1)],
                            rhs=lhs_src[:, j, :],
                            start=(j == 0), stop=(j == NT - 1))
                    hcopy(dst[:, i, :], dps[:, :])

                diff_tasks = []
                for at_sb_, lhs_, dst_ in (
                    (atq_sb, xqb_sb, d_rows[("q", 1)]),
                    (atv_sb, xvb_sb, d_rows[("v", 1)]),
                    (atq_sb, d_rows[("q", 1)], d_rows[("q", 2)]),
                    (atv_sb, d_rows[("v", 1)], d_rows[("v", 2)]),
                ):
                    for i_ in range(NT):
                        diff_tasks.append((at_sb_, lhs_, dst_, i_))
                diff_tasks = diff_tasks[::-1]  # pop from the end

                def emit_diff(n):
                    for _ in range(n):
                        if diff_tasks:
                            diffuse_tile(*diff_tasks.pop())

                with (
                    tc.tile_pool(name="psE", bufs=1, space="PSUM") as psE,
                    tc.tile_pool(name="psEa", bufs=1, space="PSUM") as psEa,
                    tc.tile_pool(name="expp", bufs=3) as expp,
                ):
                    def av_tiles(t3):
                        return [psEa.tile([32, QT3], f32, tag=f"avP{h}", name=f"av{t3}{h}")
                                for h in range(H)]

                    def flush(t3, avs):
                        for h in range(H):
                            nc.vector.tensor_copy(
                                oT_sb[64 * (h % 2):64 * (h % 2) + 32, h // 2, t3, :],
                                avs[h][:, :])

                    avs = av_tiles(0)
                    av_state["avs"] = avs
                    expair = None
                    for grp in range(3):
                        if grp > 0:
                            ln_grp(grp, tp_pool)
                        for kt in range(grp * NT, (grp + 1) * NT):
                            if kt % 2 == 0:
                                expair = expp.tile([P, 2, H, QT3], fp8, tag="ex")
                            attn_iter(0, kt, psE, expair)
                            emit_diff(2)
                    emit_diff(32)
                    flush(0, avs)
                    o_chain(0, tp_pool)
                    for t3 in (1, 2):
                        avs = av_tiles(t3)
                        av_state["avs"] = avs
                        for kt in range(LT):
                            if kt % 2 == 0:
                                expair = expp.tile([P, 2, H, QT3], fp8, tag="ex")
                            attn_iter(t3, kt, psE, expair)
                        flush(t3, avs)
                        if t3 == 1:
                            o_chain(1, tp_pool)

            # ================= epilogue =================
            with (
                tc.tile_pool(name="psF", bufs=4, space="PSUM") as psF,
                tc.tile_pool(name="psFf", bufs=1, space="PSUM") as psFf,
            ):
                o_chain(2, psF)
                # pooling: xT[d, n] = sum_q Pq^T olnT[:, 3n+q]
                xps = psFf.tile([D, CN], f32, tag="xps")
                olv = olnT_sb[:, :].rearrange("d (j s) -> d s j", s=3)
                for q in range(3):
                    nc.tensor.matmul(xps[:, :], lhsT=pq_sb[:, q, :], rhs=olv[:, q, :],
                                     start=(q == 0), stop=(q == 2))
                nc.vector.tensor_copy(xT_sb[:, :], xps[:, :])
                # conv/relu/lin/residual (1/3 pool-mean folded into conv_w3)
                x1ps = psFf.tile([DOUT, CN], f32, tag="x1ps")
                nc.tensor.matmul(x1ps[:, :], lhsT=convw_sb[:, :], rhs=xT_sb[:, :],
                                 start=True, stop=True)
                nc.scalar.activation(x1_sb[:, :], x1ps[:, :], AF.Relu,
                                     bias=convb_sb[:, :], scale=1.0)
                x2ps = psFf.tile([D, CN], f32, tag="x2ps")
                nc.tensor.matmul(x2ps[:, :], lhsT=linw_sb[:, :], rhs=x1_sb[:, :],
                                 start=True, stop=True)
                nc.vector.tensor_scalar_add(x3T_sb[:, :], x2ps[:, :], linb_sb[:, :])
                nc.vector.tensor_add(x3T_sb[:, :], x3T_sb[:, :], rest_sb[:, :])
                # rows + final LN (affine, eps 1e-5)
                for n2 in range(2):
                    tpf = psF.tile([P, D], f32, tag="tp")
                    nc.tensor.transpose(tpf[:, :], x3T_sb[:, P * n2:P * (n2 + 1)], idn[:D, :D])
                    nc.vector.tensor_copy(xr_sb[:, n2, :], tpf[:, :])
                mv3 = tmp.tile([P, 2, 2], f32, tag="mv3")
                for n2 in range(2):
                    st3 = tmp.tile([P, 6], f32, tag="bnst3")
                    nc.vector.bn_stats(st3[:, :], xr_sb[:, n2, :])
                    nc.vector.bn_aggr(mv3[:, n2, :], st3[:, :])
                ve3 = tmp.tile([P, 2], f32, tag="ve3")
                nc.vector.tensor_scalar_add(ve3[:, :], mv3[:, :, 1], 1e-5)
                rst3 = tmp.tile([P, 2], f32, tag="rst3")
                rsqrt_newton(rst3[:, :], ve3[:, :], [P, 2], "fl")
                for n2 in range(2):
                    nc.vector.tensor_scalar(
                        out=yout[:, n2, :], in0=xr_sb[:, n2, :],
                        scalar1=mv3[:, n2, 0:1], scalar2=rst3[:, n2:n2 + 1],
                        op0=OP.subtract, op1=OP.mult)
                    nc.gpsimd.tensor_mul(yout[:, n2, :], yout[:, n2, :], nw_sb[:, :])
                    nc.gpsimd.tensor_add(yout[:, n2, :], yout[:, n2, :], nb_sb[:, :])
                nc.sync.dma_start(out_d[:, :].rearrange("(t p) d -> p t d", p=P), yout[:, :, :])

    nc.finalize()
    return nc


def _prep_in_maps(inputs):
    import ml_dtypes
    bf = ml_dtypes.bfloat16

    q_x = np.asarray(inputs["q_x"], np.float32)
    v_x = np.asarray(inputs["v_x"], np.float32)
    q_adj = np.asarray(inputs["q_adj"], np.float32)
    v_adj = np.asarray(inputs["v_adj"], np.float32)
    w_qs = np.asarray(inputs["w_qs"], np.float32)
    w_ks = np.asarray(inputs["w_ks"], np.float32)
    w_vs = np.asarray(inputs["w_vs"], np.float32)
    w_fc = np.asarray(inputs["w_fc"], np.float32)
    mha_ln_w = np.asarray(inputs["mha_ln_w"], np.float32)
    mha_ln_b = np.asarray(inputs["mha_ln_b"], np.float32)
    conv_w = np.asarray(inputs["conv_w"], np.float32)
    conv_b = np.asarray(inputs["conv_b"], np.float32)
    lin_w = np.asarray(inputs["lin_w"], np.float32)
    lin_b = np.asarray(inputs["lin_b"], np.float32)
    norm_w = np.asarray(inputs["norm_w"], np.float32)
    norm_b = np.asarray(inputs["norm_b"], np.float32)

    # M_h = (Wq_h @ Wk_h^T) / sqrt(DK), head pairs side by side
    m_pair = np.zeros((D, 2 * P), np.float32)
    for h in range(H):
        m_pair[:, D * h:D * (h + 1)] = (
            w_qs[:, DK * h:DK * (h + 1)] @ w_ks[:, DK * h:DK * (h + 1)].T
        ) / np.sqrt(DK)
    # V projection: head h at cols 32h..32h+16 (16 V cols + ones col)
    wv32 = np.zeros((D + 1, P), np.float32)
    for h in range(H):
        wv32[:D, 32 * h:32 * h + DV] = w_vs[:, DV * h:DV * (h + 1)]
        wv32[D, 32 * h + DV] = 1.0
    # pooling selectors: out(n,d) = sum_q sum_c Pq[c,d] oln[3n+q, c]
    pq_w = np.zeros((D, 3, D), np.float32)
    for d in range(D):
        for s in range(STEPS):
            q, c = divmod(3 * d + s, D)
            pq_w[c, q, d] = 1.0
    conv_w3 = conv_w / 3.0

    # column permutation for A^T: col' i*128+m holds original node m*8+i
    idx = np.arange(N)
    colperm = (idx % P) * NT + idx // P

    shared = dict(
        m_pair=m_pair.astype(bf),
        wv32=wv32.astype(bf),
        wfc=w_fc.astype(bf),
        mha_w=mha_ln_w.astype(bf), mha_b=mha_ln_b.astype(bf),
        pq_w=pq_w.reshape(D, 3 * D).astype(bf),
        conv_w3=conv_w3.astype(bf), conv_b=conv_b,
        lin_w=lin_w.astype(bf), lin_b=lin_b,
        norm_w=norm_w, norm_b=norm_b,
    )

    per_batch = []
    for b in range(B):
        A, Av = q_adj[b], v_adj[b]
        A2 = A @ A
        G = np.concatenate([np.eye(N, dtype=np.float32), A, A2], axis=0)  # [3N, N]
        per_batch.append(dict(
            xqb=q_x[b].astype(bf), xvb=v_x[b].astype(bf),
            atq=np.ascontiguousarray(A.T[:, colperm]).astype(bf),
            atv=np.ascontiguousarray(Av.T[:, colperm]).astype(bf),
            G=G,
        ))

    in_maps = []
    for c in range(8):
        b, g = c // 4, c % 4
        pb = per_batch[b]
        acq = np.ascontiguousarray(pb["G"][CH * g:CH * (g + 1)].T).astype(bf)  # [N, CH]
        rest = np.ascontiguousarray(q_x[b, CN * g:CN * (g + 1)].T)             # [D, CN]
        m = dict(shared)
        m.update(xqb=pb["xqb"], xvb=pb["xvb"],
                 atq=pb["atq"], atv=pb["atv"], acq=acq, rest=rest)
        in_maps.append(m)
    return in_maps


def _run(inputs, trace=False, **kw):
    from concourse.bass_utils import run_bass_kernel_spmd

    if "nc" not in _CACHE:
        _CACHE["nc"] = _build_nc()
    nc = _CACHE["nc"]
    in_maps = _prep_in_maps(inputs)
    res = run_bass_kernel_spmd(nc, in_maps, core_ids=list(range(8)), trace=trace, **kw)
    out = np.empty((B, N, D), np.float32)
    for c in range(8):
        b, g = c // 4, c % 4
        out[b, CN * g:CN * (g + 1)] = res.results[c]["out"]
    return out, res


def kernel(**inputs) -> np.ndarray:
    out, _ = _run(inputs, trace=False)
    return out
